# revision 1
# baseline (speedup 1.0000x reference)
"""Multi-head causal attention (B=4, T=2048, D=1024, H=16) on 8 TRN2 cores.

Tensor-parallel over heads: core c computes heads {2c, 2c+1}. Each core:
  - Q', K', V' feature-major ([feat, tok]) via 512-wide PE matmuls,
  - V' -> token-major V via PE transposes, augmented with a ones column
    (fused softmax denominator),
  - S^T = K'^T Q' tiles [128 k x 512 q] (so softmax reductions over keys
    happen on the PE and P^T is directly the PV moving operand),
  - exp (no max subtraction: |S|/32 <= ~2), multiplicative 0/1 causal mask,
    diagonal blocks narrowed to their live query range,
  - per-query normalization via DVE reciprocal + GpSimd partition broadcast,
  - w_proj row-slice partial matmul, written feature-major.
Host sums the 8 partial projections and transposes back.
"""

import sys

for _p in ("/opt/trn_rl_repo",):
    if _p not in sys.path:
        sys.path.append(_p)

import numpy as np
import ml_dtypes

B, T, D = 4, 2048, 1024
H = 16
HD = D // H
NORM = float(np.sqrt(D))
N_CORES = 8
HEADS_PER_CORE = H // N_CORES          # 2
FPC = HEADS_PER_CORE * HD              # 128 features per core
QC = 512                               # query chunk
NQC = T // QC                          # 4
KB = 128                               # key block
DKC = D // 128                         # 8 contraction chunks over D

_BF16 = ml_dtypes.bfloat16

_cache = {}
_CHAIN = True


def _build():
    import concourse.bacc as bacc
    import concourse.mybir as mybir
    from concourse.tile import TileContext, add_dep_helper
    from concourse.alu_op_type import AluOpType
    from concourse.masks import make_identity

    f32 = mybir.dt.float32
    bf16 = mybir.dt.bfloat16
    EXP = mybir.ActivationFunctionType.Exp
    LN = mybir.ActivationFunctionType.Ln

    nc = bacc.Bacc("TRN2", target_bir_lowering=False, debug=False,
                   num_devices=N_CORES)

    xt = nc.dram_tensor("xt", [B, D, T], bf16, kind="ExternalInput").ap()
    w3 = nc.dram_tensor("w3", [D, 3 * FPC], bf16, kind="ExternalInput").ap()
    wp = nc.dram_tensor("wp", [FPC, D], bf16, kind="ExternalInput").ap()
    masks = nc.dram_tensor("masks", [4, KB, QC], bf16, kind="ExternalInput").ap()
    out = nc.dram_tensor("out", [B, D, T], f32, kind="ExternalOutput").ap()

    with TileContext(nc) as tc:
        with (
            tc.tile_pool(name="const", bufs=1) as cpool,
            tc.tile_pool(name="xp", bufs=12) as xpool,
            tc.tile_pool(name="qk", bufs=2) as qkpool,
            tc.tile_pool(name="vaug", bufs=20) as vpool,
            tc.tile_pool(name="pt", bufs=20) as ptpool,
            tc.tile_pool(name="y", bufs=9) as ypool,
            tc.tile_pool(name="sm", bufs=1) as smpool,
            tc.tile_pool(name="bcp", bufs=3) as bcpool,
            tc.tile_pool(name="ot", bufs=4) as otpool,
            tc.tile_pool(name="psA", bufs=2, space="PSUM") as psA,
            tc.tile_pool(name="psY", bufs=2, space="PSUM") as psY,
            tc.tile_pool(name="psO", bufs=2, space="PSUM") as psO,
        ):
            # ---- constants ----
            w3_t = []
            for kc in range(DKC):
                t = cpool.tile([128, 3 * FPC], bf16, tag=f"w3{kc}")
                nc.sync.dma_start(t[:], w3[kc * 128:(kc + 1) * 128, :])
                w3_t.append(t)
            wp_t = cpool.tile([FPC, D], bf16, tag="wp")
            nc.sync.dma_start(wp_t[:], wp[:])
            mask_t = []
            for p in range(4):
                t = cpool.tile([KB, QC], bf16, tag=f"mask{p}")
                nc.sync.dma_start(t[:], masks[p])
                mask_t.append(t)
            ident = cpool.tile([128, 128], bf16, tag="ident")
            make_identity(nc, ident[:])

            # PE warmup during the initial x DMA: keeps the HAM clock-gate
            # busy so real matmuls start at 2.4 GHz.
            psw = psO.tile([128, QC], f32, tag="pso")
            for _ in range(90):
                nc.tensor.matmul(psw[:, 0:128], lhsT=ident[:], rhs=ident[:],
                                 start=True, stop=True)

            def norm_proj(b, stash):
                # normalization chain (latency hidden under the next batch's
                # QKV matmuls) followed by the projection
                with nc.named_scope("norm"):
                    # Gather the 8 denominator rows into one [8, QC] tile via
                    # DMA (no partition-offset limits), take one partition-
                    # parallel DVE reciprocal, scatter back, broadcast,
                    # multiply on GpSimd.
                    sums8 = smpool.tile([2 * NQC, QC], f32, tag="s8")
                    for qc in range(NQC):
                        for h in range(HEADS_PER_CORE):
                            r = 2 * qc + h
                            nc.sync.dma_start(sums8[r:r + 1, :],
                                              stash[qc, h][0][:])
                    rec8 = smpool.tile([2 * NQC, QC], f32, tag="r8")
                    nc.vector.reciprocal(rec8[:], sums8[:])
                    for qc in range(NQC):
                        for h in range(HEADS_PER_CORE):
                            r = 2 * qc + h
                            rec1 = smpool.tile([1, QC], f32, tag=f"rec{r}")
                            nc.sync.dma_start(rec1[:], rec8[r:r + 1, :])
                            yu, y = stash[qc, h][1], stash[qc, h][2]
                            bc = bcpool.tile([64, QC], f32, tag="bc")
                            nc.gpsimd.partition_broadcast(bc[:], rec1[:])
                            nc.gpsimd.tensor_tensor(
                                y[HD * h:HD * (h + 1), :], yu[:], bc[:],
                                op=AluOpType.mult,
                            )
                with nc.named_scope("proj"):
                    for qc in range(NQC):
                        y = stash[qc, 0][2]
                        for mt in range(D // 128):
                            pso = psO.tile([128, QC], f32, tag="pso")
                            nc.tensor.matmul(
                                pso[:],
                                lhsT=wp_t[:, 128 * mt:128 * (mt + 1)],
                                rhs=y[:],
                                start=True, stop=True,
                            )
                            ot = otpool.tile([128, QC], f32, tag="ot")
                            if mt % 2 == 0:
                                nc.vector.tensor_copy(ot[:], pso[:])
                            else:
                                nc.scalar.copy(ot[:], pso[:])
                            nc.sync.dma_start(
                                out[b, 128 * mt:128 * (mt + 1), QC * qc:QC * (qc + 1)],
                                ot[:],
                            )

            prev = None
            for b in range(B):
                # ---- load x^T (feature-major) ----
                xp_t = []
                for kc in range(DKC):
                    t = xpool.tile([128, T], bf16, tag="xp")
                    nc.sync.dma_start(t[:], xt[b, kc * 128:(kc + 1) * 128, :])
                    xp_t.append(t)

                # ---- Q', K', V' feature-major [128, T] ----
                with nc.named_scope("qkv"):
                    qp = qkpool.tile([128, T], bf16, tag="qp")
                    kp = qkpool.tile([128, T], bf16, tag="kp")
                    vp = qkpool.tile([128, T], bf16, tag="vp")
                    for ft, dst in ((0, qp), (1, kp), (2, vp)):
                        for np2 in range(NQC // 2):
                            ps = psA.tile([128, 2 * QC], f32, tag="ps")
                            for half in range(2):
                                ntk = 2 * np2 + half
                                for kc in range(DKC):
                                    nc.tensor.matmul(
                                        ps[:, QC * half:QC * (half + 1)],
                                        lhsT=w3_t[kc][:, 128 * ft:128 * (ft + 1)],
                                        rhs=xp_t[kc][:, QC * ntk:QC * (ntk + 1)],
                                        start=(kc == 0), stop=(kc == DKC - 1),
                                    )
                            nc.vector.tensor_copy(
                                dst[:, 2 * QC * np2:2 * QC * (np2 + 1)], ps[:])

                # previous batch's normalization + projection, emitted here
                # so the PE stream never waits on the norm chain
                if prev is not None:
                    norm_proj(*prev)

                # ---- V' -> token-major V, augmented with ones cols ----
                # layout: [v_h0(64) | ones | v_h1(64) | ones]
                with nc.named_scope("vtrans"):
                    vaug_t = []
                    for tk in range(T // 128):
                        ps = psY.tile([128, FPC], bf16, tag="psy")
                        nc.tensor.transpose(
                            ps[:], vp[:, 128 * tk:128 * (tk + 1)], ident[:]
                        )
                        va = vpool.tile([128, 2 * HD + 2], bf16, tag="vaug")
                        nc.vector.tensor_copy(va[:, 0:HD], ps[:, 0:HD])
                        nc.vector.tensor_copy(va[:, HD + 1:2 * HD + 1], ps[:, HD:2 * HD])
                        nc.gpsimd.memset(va[:, HD:HD + 1], 1.0)
                        nc.gpsimd.memset(va[:, 2 * HD + 1:2 * HD + 2], 1.0)
                        vaug_t.append(va)

                # ---- attention per query chunk ----
                stash = {}
                exp_insts = []

                def score_pair(qc, h, kb2, pts):
                    nkb = (qc + 1) * (QC // KB)
                    kbA, kbB = 2 * kb2, 2 * kb2 + 1
                    j0A = max(0, KB * (kbA - qc * 4))
                    j0B = max(0, KB * (kbB - qc * 4))
                    pss = psA.tile([128, 2 * QC], f32, tag="ps")
                    for off, kb, j0 in ((0, kbA, j0A), (QC, kbB, j0B)):
                        nc.tensor.matmul(
                            pss[:, off + j0:off + QC],
                            lhsT=kp[HD * h:HD * (h + 1), KB * kb:KB * (kb + 1)],
                            rhs=qp[HD * h:HD * (h + 1), QC * qc + j0:QC * (qc + 1)],
                            start=True, stop=True,
                        )
                    pt = ptpool.tile([KB, 2 * QC], bf16, tag="pt")
                    if j0B == 0:
                        exp_insts.append(nc.scalar.activation(
                            pt[:, j0A:2 * QC], pss[:, j0A:2 * QC],
                            EXP, scale=1.0 / NORM).ins)
                    else:
                        exp_insts.append(nc.scalar.activation(
                            pt[:, j0A:QC], pss[:, j0A:QC],
                            EXP, scale=1.0 / NORM).ins)
                        exp_insts.append(nc.scalar.activation(
                            pt[:, QC + j0B:2 * QC], pss[:, QC + j0B:2 * QC],
                            EXP, scale=1.0 / NORM).ins)
                    for off, kb, j0 in ((0, kbA, j0A), (QC, kbB, j0B)):
                        p = kb - qc * 4
                        if p >= 0:
                            nc.vector.tensor_tensor(
                                pt[:, off + j0:off + QC],
                                pt[:, off + j0:off + QC],
                                mask_t[p][:, j0:QC],
                                op=AluOpType.mult,
                            )
                        pts[h, kb] = (pt, off, j0)

                def pv_step(qc, h, kb, psy, pts, start, stop):
                    pt, off, j0 = pts[h, kb]
                    nc.tensor.matmul(
                        psy[0:HD + 1, j0:QC],
                        lhsT=vaug_t[kb][:, (HD + 1) * h:(HD + 1) * (h + 1)],
                        rhs=pt[:, off + j0:off + QC],
                        start=start, stop=stop,
                    )

                def pv_drain(qc, h, psy, y):
                    srow = smpool.tile([1, QC], f32, tag=f"srow{2 * qc + h}")
                    nc.vector.tensor_copy(srow[:], psy[HD:HD + 1, :])
                    yu = smpool.tile([64, QC], f32, tag=f"yu{2 * qc + h}")
                    nc.vector.tensor_copy(yu[:], psy[0:HD, :])
                    stash[qc, h] = (srow, yu, y)

                for qc in range(NQC):
                    nkb = (qc + 1) * (QC // KB)
                    y = ypool.tile([FPC, QC], bf16, tag="y")
                    pts = {}
                    with nc.named_scope("score"):
                        # h0 scores first, then h1 scores interleaved with
                        # h0 PV so the PE always has a runnable matmul while
                        # ACT works through the exps
                        for kb2 in range(nkb // 2):
                            score_pair(qc, 0, kb2, pts)
                        # non-diagonal key blocks accumulate first so the
                        # masked diagonal blocks' DVE multiplies have time to
                        # land before PV needs them
                        kb_order = [kb for kb in range(nkb) if kb < qc * 4] + \
                                   [kb for kb in range(nkb) if kb >= qc * 4]
                        psy0 = psY.tile([HD + 1, QC], f32, tag="psy")
                        for kb2 in range(nkb // 2):
                            score_pair(qc, 1, kb2, pts)
                            pv_step(qc, 0, kb_order[2 * kb2], psy0, pts,
                                    2 * kb2 == 0, False)
                            pv_step(qc, 0, kb_order[2 * kb2 + 1], psy0, pts,
                                    False, 2 * kb2 + 2 == nkb)
                        pv_drain(qc, 0, psy0, y)
                    with nc.named_scope("pv"):
                        psy1 = psY.tile([HD + 1, QC], f32, tag="psy")
                        for i, kb in enumerate(kb_order):
                            pv_step(qc, 1, kb, psy1, pts, i == 0, i == nkb - 1)
                        pv_drain(qc, 1, psy1, y)

                prev = (b, stash)

            norm_proj(*prev)

    nc.compile()
    return nc


def _get_nc():
    if "nc" not in _cache:
        _cache["nc"] = _build()
    return _cache["nc"]


def _make_masks():
    i = np.arange(KB)[:, None]
    j = np.arange(QC)[None, :]
    m = np.zeros((4, KB, QC), dtype=np.float32)
    for p in range(4):
        m[p] = (j >= (KB * p + i)).astype(np.float32)
    return m.astype(_BF16)


def shard_inputs(x, w_qkv, w_proj):
    xt = np.ascontiguousarray(np.asarray(x, dtype=np.float32).transpose(0, 2, 1))
    xt = xt.astype(_BF16)
    w_qkv = np.asarray(w_qkv, dtype=np.float32)
    w_proj = np.asarray(w_proj, dtype=np.float32)
    masks = _make_masks()
    in_maps = []
    for c in range(N_CORES):
        qcols = slice(FPC * c, FPC * (c + 1))
        kcols = slice(D + FPC * c, D + FPC * (c + 1))
        vcols = slice(2 * D + FPC * c, 2 * D + FPC * (c + 1))
        w3_c = np.concatenate(
            [w_qkv[:, qcols], w_qkv[:, kcols], w_qkv[:, vcols]], axis=1)
        in_maps.append({
            "xt": xt,
            "w3": np.ascontiguousarray(w3_c).astype(_BF16),
            "wp": np.ascontiguousarray(w_proj[FPC * c:FPC * (c + 1), :]).astype(_BF16),
            "masks": masks,
        })
    return in_maps


def unshard(results):
    total = results[0]["out"].astype(np.float32)
    for r in results[1:]:
        total += r["out"]
    return np.ascontiguousarray(total.transpose(0, 2, 1))


def run(inputs, trace=False, **kw):
    from concourse.bass_utils import run_bass_kernel_spmd

    nc = _get_nc()
    in_maps = shard_inputs(inputs["x"], inputs["w_qkv"], inputs["w_proj"])
    res = run_bass_kernel_spmd(nc, in_maps, core_ids=list(range(N_CORES)),
                               trace=trace, **kw)
    return unshard(res.results), res


def kernel(**inputs):
    out, _ = run(inputs, trace=False)
    return out



# revision 3
# speedup vs baseline: 1.6364x; 1.6364x over previous
"""Multi-head causal attention (B=4, T=2048, D=1024, H=16) on 8 TRN2 cores.

Tensor-parallel over heads: core c computes heads {2c, 2c+1}. Redesign vs
the previous version:
  - Score matmuls for h0 (PE rows 0-63) and h1 (rows 64-127) are emitted
    back-to-back into separate PSUM banks so the row-tiled pairs execute
    concurrently (~2x score throughput).
  - V is computed directly token-major (lhsT = x^T token slice), so there
    are no PE transposes and no transpose->copy->memset chain.
  - vaug ones-columns are persistent tiles memset once at startup (the old
    per-batch gpsimd memsets stalled PV ~30us/batch behind the norm chain).
  - Norm chain: DVE copy of the denominator row, reciprocal_approx_fast,
    gpsimd partition_broadcast, DVE multiply into y (bf16).
  - Proj drains on DVE as bf16; output DMA'd bf16 and summed on host.
  - Emission is software-pipelined: section b emits QKV(b) interleaved with
    attention of batch b-1 (scores qc0/1 during QKV, PV/proj later), and
    proj(b-1, qc3) is deferred into section b+1.
"""

import sys

for _p in ("/opt/trn_rl_repo",):
    if _p not in sys.path:
        sys.path.append(_p)

import numpy as np
import ml_dtypes

B, T, D = 4, 2048, 1024
H = 16
HD = D // H
NORM = float(np.sqrt(D))
N_CORES = 8
HEADS_PER_CORE = H // N_CORES          # 2
FPC = HEADS_PER_CORE * HD              # 128 features per core
QC = 512                               # query chunk
NQC = T // QC                          # 4
KB = 128                               # key block
DKC = D // 128                         # 8 contraction chunks over D
NTB = T // 128                         # 16 token blocks

_BF16 = ml_dtypes.bfloat16

_cache = {}

N_WARM = 60


def _build():
    import concourse.bacc as bacc
    import concourse.mybir as mybir
    from concourse.tile import TileContext
    from concourse.alu_op_type import AluOpType

    f32 = mybir.dt.float32
    bf16 = mybir.dt.bfloat16
    EXP = mybir.ActivationFunctionType.Exp

    nc = bacc.Bacc("TRN2", target_bir_lowering=False, debug=False,
                   num_devices=N_CORES)

    xt = nc.dram_tensor("xt", [B, D, T], bf16, kind="ExternalInput").ap()
    w3 = nc.dram_tensor("w3", [D, 3 * FPC], bf16, kind="ExternalInput").ap()
    wp = nc.dram_tensor("wp", [FPC, D], bf16, kind="ExternalInput").ap()
    masks = nc.dram_tensor("masks", [4, KB, QC], bf16, kind="ExternalInput").ap()
    out = nc.dram_tensor("out", [B, D, T], bf16, kind="ExternalOutput").ap()

    with TileContext(nc) as tc:
        with (
            tc.tile_pool(name="const", bufs=1) as cpool,
            tc.tile_pool(name="xp", bufs=12) as xpool,
            tc.tile_pool(name="qk", bufs=2) as qkpool,
            tc.tile_pool(name="pt", bufs=60) as ptpool,
            tc.tile_pool(name="y", bufs=6) as ypool,
            tc.tile_pool(name="sm", bufs=3) as smpool,
            tc.tile_pool(name="bcp", bufs=2) as bcpool,
            tc.tile_pool(name="ot", bufs=4) as otpool,
            tc.tile_pool(name="psA", bufs=4, space="PSUM") as psA,
            tc.tile_pool(name="psY", bufs=2, space="PSUM") as psY,
            tc.tile_pool(name="psO", bufs=2, space="PSUM") as psO,
        ):
            # ---- constants ----
            w3_t = []
            for kc in range(DKC):
                t = cpool.tile([128, 3 * FPC], bf16, tag=f"w3{kc}")
                nc.sync.dma_start(t[:], w3[kc * 128:(kc + 1) * 128, :])
                w3_t.append(t)
            wp_t = cpool.tile([FPC, D], bf16, tag="wp")
            nc.sync.dma_start(wp_t[:], wp[:])
            mask_t = []
            for p in range(4):
                t = cpool.tile([KB, QC], bf16, tag=f"mask{p}")
                nc.sync.dma_start(t[:], masks[p])
                mask_t.append(t)

            # persistent vaug tiles: [v_h0 64 | ones | v_h1 64 | ones],
            # double-buffered across batches; ones written once here.
            vaug = [[], []]
            for g in range(2):
                for tb in range(NTB):
                    va = cpool.tile([128, 2 * HD + 2], bf16, tag=f"va{g}_{tb}")
                    nc.gpsimd.memset(va[:, HD:HD + 1], 1.0)
                    nc.gpsimd.memset(va[:, 2 * HD + 1:2 * HD + 2], 1.0)
                    vaug[g].append(va)

            # PE warmup on a memset tile (no DMA dependency): keeps the HAM
            # clock-gate busy during the initial x DMA.
            wt = cpool.tile([128, 128], bf16, tag="warm")
            nc.vector.memset(wt[:], 0.25)
            psw = psO.tile([128, QC], f32, tag="pso")
            for _ in range(N_WARM):
                nc.tensor.matmul(psw[:, 0:128], lhsT=wt[:], rhs=wt[:],
                                 start=True, stop=True)

            # ---- mutable cross-section state ----
            xp_t = {}      # b -> [8 tiles]
            qkp = {}       # b -> (qp, kp)
            pts = {}       # (a, qc, h, kb) -> (pt tile, j0)
            y_tiles = {}   # (a, qc) -> y tile

            def prefetch_x(b):
                ts = []
                for kc in range(DKC):
                    t = xpool.tile([128, T], bf16, tag="xp")
                    nc.sync.dma_start(t[:], xt[b, kc * 128:(kc + 1) * 128, :])
                    ts.append(t)
                xp_t[b] = ts

            # ---- QKV units ----
            def emit_qk(b, ft, c):
                # ft: 0=Q, 1=K ; c: 512-token chunk
                if b not in qkp:
                    qp = qkpool.tile([128, T], bf16, tag="qp")
                    kp = qkpool.tile([128, T], bf16, tag="kp")
                    qkp[b] = (qp, kp)
                dst = qkp[b][ft]
                ps = psA.tile([128, QC], f32, tag="ps")
                for kc in range(DKC):
                    nc.tensor.matmul(
                        ps[:],
                        lhsT=w3_t[kc][:, 128 * ft:128 * (ft + 1)],
                        rhs=xp_t[b][kc][:, QC * c:QC * (c + 1)],
                        start=(kc == 0), stop=(kc == DKC - 1),
                    )
                nc.vector.tensor_copy(dst[:, QC * c:QC * (c + 1)], ps[:])

            def emit_v(b, c, half, state={}):
                # token-major V for token blocks [4c+2*half, 4c+2*half+1]
                if half == 0:
                    state[b, c] = psA.tile([128, QC], f32, tag="ps", name="psv")
                ps = state[b, c]
                for j in (2 * half, 2 * half + 1):
                    tb = 4 * c + j
                    for kc in range(DKC):
                        nc.tensor.matmul(
                            ps[:, 128 * j:128 * (j + 1)],
                            lhsT=xp_t[b][kc][:, 128 * tb:128 * (tb + 1)],
                            rhs=w3_t[kc][:, 256:384],
                            start=(kc == 0), stop=(kc == DKC - 1),
                        )
                for j in (2 * half, 2 * half + 1):
                    tb = 4 * c + j
                    va = vaug[b % 2][tb]
                    nc.vector.tensor_copy(va[:, 0:HD],
                                          ps[:, 128 * j:128 * j + HD])
                    nc.vector.tensor_copy(va[:, HD + 1:2 * HD + 1],
                                          ps[:, 128 * j + HD:128 * (j + 1)])

            # ---- attention units ----
            def emit_slot(a, qc, kb):
                # one key block, both heads: two concurrent row-tiled MMs
                # into separate PSUM banks, exp on ACT, mask on DVE.
                j0 = max(0, KB * (kb - 4 * qc))
                p = kb - 4 * qc
                qp, kp = qkp[a]
                ps_h = []
                for h in range(2):
                    ps = psA.tile([128, QC], f32, tag="ps")
                    nc.tensor.matmul(
                        ps[:, j0:QC],
                        lhsT=kp[HD * h:HD * (h + 1), KB * kb:KB * (kb + 1)],
                        rhs=qp[HD * h:HD * (h + 1), QC * qc + j0:QC * (qc + 1)],
                        start=True, stop=True,
                    )
                    ps_h.append(ps)
                for h in range(2):
                    pt = ptpool.tile([KB, QC], bf16, tag="pt")
                    nc.scalar.activation(pt[:, j0:QC], ps_h[h][:, j0:QC],
                                         EXP, scale=1.0 / NORM)
                    if p >= 0:
                        nc.vector.tensor_tensor(
                            pt[:, j0:QC], pt[:, j0:QC], mask_t[p][:, j0:QC],
                            op=AluOpType.mult,
                        )
                    pts[a, qc, h, kb] = (pt, j0)

            def emit_pv(a, qc, h, kbs, psy, nkb, state={}):
                for kb in kbs:
                    pt, j0 = pts.pop((a, qc, h, kb))
                    i = state.get((a, qc, h), 0)
                    nc.tensor.matmul(
                        psy[0:HD + 1, j0:QC],
                        lhsT=vaug[a % 2][kb][:, (HD + 1) * h:(HD + 1) * (h + 1)],
                        rhs=pt[:, j0:QC],
                        start=(i == 0), stop=(i == nkb - 1),
                    )
                    state[a, qc, h] = i + 1

            def emit_drain(a, qc, h, psy):
                if (a, qc) not in y_tiles:
                    y_tiles[a, qc] = ypool.tile([FPC, QC], bf16, tag="y", name="y")
                y = y_tiles[a, qc]
                srow = smpool.tile([1, QC], f32, tag=f"srow{h}")
                nc.vector.tensor_copy(srow[:], psy[HD:HD + 1, :])
                yu = smpool.tile([HD, QC], f32, tag=f"yu{h}")
                nc.vector.tensor_copy(yu[:], psy[0:HD, :])
                rec = smpool.tile([1, QC], f32, tag=f"rec{h}")
                nc.vector.reciprocal_approx_fast(rec[:], srow[:])
                bc = bcpool.tile([HD, QC], f32, tag=f"bc{h}")
                nc.gpsimd.partition_broadcast(bc[:], rec[:])
                nc.vector.tensor_tensor(y[HD * h:HD * (h + 1), :], yu[:],
                                        bc[:], op=AluOpType.mult)

            def emit_proj(a, qc, mts):
                y = y_tiles[a, qc]
                for mt in mts:
                    pso = psO.tile([128, QC], f32, tag="pso")
                    nc.tensor.matmul(
                        pso[:],
                        lhsT=wp_t[:, 128 * mt:128 * (mt + 1)],
                        rhs=y[:],
                        start=True, stop=True,
                    )
                    ot = otpool.tile([128, QC], bf16, tag="ot")
                    nc.vector.tensor_copy(ot[:], pso[:])
                    nc.sync.dma_start(
                        out[a, 128 * mt:128 * (mt + 1), QC * qc:QC * (qc + 1)],
                        ot[:],
                    )

            def kb_order(qc):
                nkb = 4 * (qc + 1)
                return ([kb for kb in range(nkb) if kb < 4 * qc] +
                        [kb for kb in range(nkb) if kb >= 4 * qc])

            def attn_units(a):
                # ordered attention stream for batch a; yields callables.
                units = []

                def slot_u(qc, kb):
                    units.append(lambda: emit_slot(a, qc, kb))

                psy_tiles = {}

                def pv_u(qc, h, kbs):
                    def f():
                        if (qc, h) not in psy_tiles:
                            psy_tiles[qc, h] = psY.tile([HD + 1, QC], f32,
                                                        tag="psy", name="psy")
                        emit_pv(a, qc, h, kbs, psy_tiles[qc, h], 4 * (qc + 1))
                    units.append(f)

                def drain_u(qc, h):
                    units.append(lambda: emit_drain(a, qc, h, psy_tiles[qc, h]))

                def proj_u(qc, mts):
                    units.append(lambda: emit_proj(a, qc, list(mts)))

                # phase 0: scores for chunks 0 and 1 (run during QKV(a+1))
                for kb in range(4):
                    slot_u(0, kb)
                for kb in range(8):
                    slot_u(1, kb)
                # phase 1: pv(0) + scores(2)
                ko0 = kb_order(0)
                pv_u(0, 0, ko0[:2]); slot_u(2, 0); slot_u(2, 1)
                pv_u(0, 0, ko0[2:]); slot_u(2, 2); slot_u(2, 3)
                drain_u(0, 0)
                pv_u(0, 1, ko0[:2]); slot_u(2, 4); slot_u(2, 5)
                pv_u(0, 1, ko0[2:]); slot_u(2, 6); slot_u(2, 7)
                drain_u(0, 1)
                slot_u(2, 8); slot_u(2, 9); slot_u(2, 10); slot_u(2, 11)
                # phase 2: pv(1) + scores(3) + proj(0)
                ko1 = kb_order(1)
                pv_u(1, 0, ko1[:4]); slot_u(3, 0); slot_u(3, 1)
                pv_u(1, 0, ko1[4:]); slot_u(3, 2); slot_u(3, 3)
                drain_u(1, 0)
                pv_u(1, 1, ko1[:4]); slot_u(3, 4); slot_u(3, 5)
                pv_u(1, 1, ko1[4:]); slot_u(3, 6); slot_u(3, 7)
                drain_u(1, 1)
                proj_u(0, range(0, 4)); slot_u(3, 8); slot_u(3, 9)
                proj_u(0, range(4, 8)); slot_u(3, 10); slot_u(3, 11)
                # phase 3: pv(2) + proj(1)
                ko2 = kb_order(2)
                pv_u(2, 0, ko2[:4]); slot_u(3, 12); slot_u(3, 13)
                pv_u(2, 0, ko2[4:8]); slot_u(3, 14); slot_u(3, 15)
                pv_u(2, 0, ko2[8:])
                drain_u(2, 0)
                pv_u(2, 1, ko2[:4]); proj_u(1, range(0, 4))
                pv_u(2, 1, ko2[4:8]); proj_u(1, range(4, 8))
                pv_u(2, 1, ko2[8:])
                drain_u(2, 1)
                # phase 4: pv(3) + proj(2)
                ko3 = kb_order(3)
                pv_u(3, 0, ko3[:4]); proj_u(2, range(0, 4))
                pv_u(3, 0, ko3[4:8]); proj_u(2, range(4, 8))
                pv_u(3, 0, ko3[8:12])
                pv_u(3, 0, ko3[12:])
                drain_u(3, 0)
                pv_u(3, 1, ko3[:4])
                pv_u(3, 1, ko3[4:8])
                pv_u(3, 1, ko3[8:12])
                pv_u(3, 1, ko3[12:])
                drain_u(3, 1)
                # proj(3) deferred to the next section
                return units

            def qkv_units(b):
                units = []
                for c in range(NQC):
                    units.append(lambda c=c: emit_qk(b, 0, c))
                    units.append(lambda c=c: emit_qk(b, 1, c))
                    units.append(lambda c=c: emit_v(b, c, 0))
                    units.append(lambda c=c: emit_v(b, c, 1))
                return units

            def merge(fill, attn):
                # interleave: lead with 2 fillers, then spread the rest
                # evenly through the attention stream.
                seq = []
                lead = fill[:2]
                rest = fill[2:]
                seq += lead
                if not attn:
                    return seq + rest
                if not rest:
                    return seq + attn
                stride = max(1, len(attn) // len(rest))
                ai = 0
                for i, f in enumerate(rest):
                    nxt = min(len(attn), (i + 1) * stride)
                    seq += attn[ai:nxt]
                    seq.append(f)
                    ai = nxt
                seq += attn[ai:]
                return seq

            # ---- sections ----
            prefetch_x(0)
            prefetch_x(1)
            for b in range(B + 1):
                if b >= 2 and b < B + 1:
                    pass
                fill = []
                if b < B:
                    fill += qkv_units(b)
                if b >= 2:
                    a2 = b - 2
                    fill.append(lambda a2=a2: emit_proj(a2, 3, [0, 1, 2, 3]))
                    fill.append(lambda a2=a2: emit_proj(a2, 3, [4, 5, 6, 7]))
                attn = attn_units(b - 1) if b >= 1 else []
                for u in merge(fill, attn):
                    u()
                if b + 2 <= B - 1:
                    prefetch_x(b + 2)
            # tail: proj(B-1, 3)
            emit_proj(B - 1, 3, list(range(8)))

    nc.compile()
    return nc


def _get_nc():
    if "nc" not in _cache:
        _cache["nc"] = _build()
    return _cache["nc"]


def _make_masks():
    i = np.arange(KB)[:, None]
    j = np.arange(QC)[None, :]
    m = np.zeros((4, KB, QC), dtype=np.float32)
    for p in range(4):
        m[p] = (j >= (KB * p + i)).astype(np.float32)
    return m.astype(_BF16)


def shard_inputs(x, w_qkv, w_proj):
    xt = np.ascontiguousarray(np.asarray(x, dtype=np.float32).transpose(0, 2, 1))
    xt = xt.astype(_BF16)
    w_qkv = np.asarray(w_qkv, dtype=np.float32)
    w_proj = np.asarray(w_proj, dtype=np.float32)
    masks = _make_masks()
    in_maps = []
    for c in range(N_CORES):
        qcols = slice(FPC * c, FPC * (c + 1))
        kcols = slice(D + FPC * c, D + FPC * (c + 1))
        vcols = slice(2 * D + FPC * c, 2 * D + FPC * (c + 1))
        w3_c = np.concatenate(
            [w_qkv[:, qcols], w_qkv[:, kcols], w_qkv[:, vcols]], axis=1)
        in_maps.append({
            "xt": xt,
            "w3": np.ascontiguousarray(w3_c).astype(_BF16),
            "wp": np.ascontiguousarray(w_proj[FPC * c:FPC * (c + 1), :]).astype(_BF16),
            "masks": masks,
        })
    return in_maps


def unshard(results):
    total = results[0]["out"].astype(np.float32)
    for r in results[1:]:
        total += r["out"].astype(np.float32)
    return np.ascontiguousarray(total.transpose(0, 2, 1))


def run(inputs, trace=False, **kw):
    from concourse.bass_utils import run_bass_kernel_spmd

    nc = _get_nc()
    in_maps = shard_inputs(inputs["x"], inputs["w_qkv"], inputs["w_proj"])
    res = run_bass_kernel_spmd(nc, in_maps, core_ids=list(range(N_CORES)),
                               trace=trace, **kw)
    return unshard(res.results), res


def kernel(**inputs):
    out, _ = run(inputs, trace=False)
    return out


# revision 10
# speedup vs baseline: 1.6386x; 1.0014x over previous
"""Multi-head causal attention (B=4, T=2048, D=1024, H=16) on 8 TRN2 cores.

Tensor-parallel over heads: core c computes heads {2c, 2c+1}. Redesign vs
the previous version:
  - Score matmuls for h0 (PE rows 0-63) and h1 (rows 64-127) are emitted
    back-to-back into separate PSUM banks so the row-tiled pairs execute
    concurrently (~2x score throughput).
  - V is computed directly token-major (lhsT = x^T token slice), so there
    are no PE transposes and no transpose->copy->memset chain.
  - vaug ones-columns are persistent tiles memset once at startup (the old
    per-batch gpsimd memsets stalled PV ~30us/batch behind the norm chain).
  - Norm chain: DVE copy of the denominator row, reciprocal_approx_fast,
    gpsimd partition_broadcast, DVE multiply into y (bf16).
  - Proj drains on DVE as bf16; output DMA'd bf16 and summed on host.
  - Emission is software-pipelined: section b emits QKV(b) interleaved with
    attention of batch b-1 (scores qc0/1 during QKV, PV/proj later), and
    proj(b-1, qc3) is deferred into section b+1.
"""

import sys

for _p in ("/opt/trn_rl_repo",):
    if _p not in sys.path:
        sys.path.append(_p)

import numpy as np
import ml_dtypes

B, T, D = 4, 2048, 1024
H = 16
HD = D // H
NORM = float(np.sqrt(D))
N_CORES = 8
HEADS_PER_CORE = H // N_CORES          # 2
FPC = HEADS_PER_CORE * HD              # 128 features per core
QC = 512                               # query chunk
NQC = T // QC                          # 4
KB = 128                               # key block
DKC = D // 128                         # 8 contraction chunks over D
NTB = T // 128                         # 16 token blocks

_BF16 = ml_dtypes.bfloat16

_cache = {}

N_WARM = 140


def _build():
    import concourse.bacc as bacc
    import concourse.mybir as mybir
    from concourse.tile import TileContext
    from concourse.alu_op_type import AluOpType

    f32 = mybir.dt.float32
    bf16 = mybir.dt.bfloat16
    EXP = mybir.ActivationFunctionType.Exp

    nc = bacc.Bacc("TRN2", target_bir_lowering=False, debug=False,
                   num_devices=N_CORES)

    xt = nc.dram_tensor("xt", [B, D, T], bf16, kind="ExternalInput").ap()
    w3 = nc.dram_tensor("w3", [D, 3 * FPC], bf16, kind="ExternalInput").ap()
    wp = nc.dram_tensor("wp", [FPC, D], bf16, kind="ExternalInput").ap()
    masks = nc.dram_tensor("masks", [4, KB, QC], bf16, kind="ExternalInput").ap()
    out = nc.dram_tensor("out", [B, D, T], bf16, kind="ExternalOutput").ap()

    with TileContext(nc) as tc:
        with (
            tc.tile_pool(name="const", bufs=1) as cpool,
            tc.tile_pool(name="xp", bufs=12) as xpool,
            tc.tile_pool(name="qk", bufs=2) as qkpool,
            tc.tile_pool(name="pt", bufs=60) as ptpool,
            tc.tile_pool(name="y", bufs=6) as ypool,
            tc.tile_pool(name="sm", bufs=3) as smpool,
            tc.tile_pool(name="bcp", bufs=2) as bcpool,
            tc.tile_pool(name="ot", bufs=4) as otpool,
            tc.tile_pool(name="psA", bufs=4, space="PSUM") as psA,
            tc.tile_pool(name="psY", bufs=2, space="PSUM") as psY,
            tc.tile_pool(name="psO", bufs=2, space="PSUM") as psO,
        ):
            # ---- constants ----
            w3_t = []
            for kc in range(DKC):
                t = cpool.tile([128, 3 * FPC], bf16, tag=f"w3{kc}")
                nc.sync.dma_start(t[:], w3[kc * 128:(kc + 1) * 128, :])
                w3_t.append(t)
            wp_t = cpool.tile([FPC, D], bf16, tag="wp")
            nc.sync.dma_start(wp_t[:], wp[:])
            mask_t = []
            for p in range(4):
                t = cpool.tile([KB, QC], bf16, tag=f"mask{p}")
                nc.sync.dma_start(t[:], masks[p])
                mask_t.append(t)

            # persistent vaug tiles: [v_h0 64 | ones | v_h1 64 | ones],
            # double-buffered across batches; ones written once here.
            vaug = [[], []]
            for g in range(2):
                for tb in range(NTB):
                    va = cpool.tile([128, 2 * HD + 2], bf16, tag=f"va{g}_{tb}")
                    nc.gpsimd.memset(va[:, HD:HD + 1], 1.0)
                    nc.gpsimd.memset(va[:, 2 * HD + 1:2 * HD + 2], 1.0)
                    vaug[g].append(va)

            # PE warmup on a memset tile (no DMA dependency): keeps the HAM
            # clock-gate busy during the initial x DMA.
            wt = cpool.tile([128, 128], bf16, tag="warm")
            nc.vector.memset(wt[:], 0.25)
            psw = psO.tile([128, QC], f32, tag="pso")
            for _ in range(N_WARM):
                nc.tensor.matmul(psw[:, 0:128], lhsT=wt[:], rhs=wt[:],
                                 start=True, stop=True)

            # ---- mutable cross-section state ----
            xp_t = {}      # b -> [8 tiles]
            qkp = {}       # b -> (qp, kp)
            pts = {}       # (a, qc, h, kb) -> (pt tile, j0)
            y_tiles = {}   # (a, qc) -> y tile

            def prefetch_x(b):
                ts = []
                for kc in range(DKC):
                    t = xpool.tile([128, T], bf16, tag="xp")
                    nc.sync.dma_start(t[:], xt[b, kc * 128:(kc + 1) * 128, :])
                    ts.append(t)
                xp_t[b] = ts

            # ---- QKV units ----
            # Each chunk c is emitted as two interleaved parts so the short
            # N=128 token-major V matmuls hide their weight loads under the
            # long N=512 Q/K streams:
            #   part 0: Q[kc] + V(tb 4c+0)[kc] + V(tb 4c+1)[kc]  for kc=0..7
            #   part 1: K[kc] + V(tb 4c+2)[kc] + V(tb 4c+3)[kc]
            def emit_qkv_part(b, c, part, state={}):
                if b not in qkp:
                    qp = qkpool.tile([128, T], bf16, tag="qp")
                    kp = qkpool.tile([128, T], bf16, tag="kp")
                    qkp[b] = (qp, kp)
                dst = qkp[b][part]
                ps = psA.tile([128, QC], f32, tag="ps")
                if part == 0:
                    state[b, c] = psA.tile([128, QC], f32, tag="ps", name="psv")
                psv = state[b, c]
                tbs = (4 * c + 2 * part, 4 * c + 2 * part + 1)
                for kc in range(DKC):
                    nc.tensor.matmul(
                        ps[:],
                        lhsT=w3_t[kc][:, 128 * part:128 * (part + 1)],
                        rhs=xp_t[b][kc][:, QC * c:QC * (c + 1)],
                        start=(kc == 0), stop=(kc == DKC - 1),
                    )
                for j, tb in enumerate(tbs):
                    jj = 2 * part + j
                    for kc in range(DKC):
                        nc.tensor.matmul(
                            psv[:, 128 * jj:128 * (jj + 1)],
                            lhsT=xp_t[b][kc][:, 128 * tb:128 * (tb + 1)],
                            rhs=w3_t[kc][:, 256:384],
                            start=(kc == 0), stop=(kc == DKC - 1),
                        )
                nc.vector.tensor_copy(dst[:, QC * c:QC * (c + 1)], ps[:])
                for j, tb in enumerate(tbs):
                    jj = 2 * part + j
                    va = vaug[b % 2][tb]
                    nc.vector.tensor_copy(va[:, 0:HD],
                                          psv[:, 128 * jj:128 * jj + HD])
                    nc.vector.tensor_copy(va[:, HD + 1:2 * HD + 1],
                                          psv[:, 128 * jj + HD:128 * (jj + 1)])

            # ---- attention units ----
            def emit_slot(a, qc, kb):
                # one key block, both heads: two concurrent row-tiled MMs
                # into separate PSUM banks, exp on ACT, mask on DVE.
                j0 = max(0, KB * (kb - 4 * qc))
                p = kb - 4 * qc
                qp, kp = qkp[a]
                ps_h = []
                for h in range(2):
                    ps = psA.tile([128, QC], f32, tag="ps")
                    nc.tensor.matmul(
                        ps[:, j0:QC],
                        lhsT=kp[HD * h:HD * (h + 1), KB * kb:KB * (kb + 1)],
                        rhs=qp[HD * h:HD * (h + 1), QC * qc + j0:QC * (qc + 1)],
                        start=True, stop=True,
                    )
                    ps_h.append(ps)
                for h in range(2):
                    pt = ptpool.tile([KB, QC], bf16, tag="pt")
                    nc.scalar.activation(pt[:, j0:QC], ps_h[h][:, j0:QC],
                                         EXP, scale=1.0 / NORM)
                    if p >= 0:
                        nc.vector.tensor_tensor(
                            pt[:, j0:QC], pt[:, j0:QC], mask_t[p][:, j0:QC],
                            op=AluOpType.mult,
                        )
                    pts[a, qc, h, kb] = (pt, j0)

            def emit_pv(a, qc, h, kbs, psy, nkb, state={}):
                for kb in kbs:
                    pt, j0 = pts.pop((a, qc, h, kb))
                    i = state.get((a, qc, h), 0)
                    nc.tensor.matmul(
                        psy[0:HD + 1, j0:QC],
                        lhsT=vaug[a % 2][kb][:, (HD + 1) * h:(HD + 1) * (h + 1)],
                        rhs=pt[:, j0:QC],
                        start=(i == 0), stop=(i == nkb - 1),
                    )
                    state[a, qc, h] = i + 1

            def emit_drain(a, qc, h, psy):
                if (a, qc) not in y_tiles:
                    y_tiles[a, qc] = ypool.tile([FPC, QC], bf16, tag="y", name="y")
                y = y_tiles[a, qc]
                srow = smpool.tile([1, QC], f32, tag=f"srow{h}")
                nc.vector.tensor_copy(srow[:], psy[HD:HD + 1, :])
                yu = smpool.tile([HD, QC], f32, tag=f"yu{h}")
                nc.vector.tensor_copy(yu[:], psy[0:HD, :])
                rec = smpool.tile([1, QC], f32, tag=f"rec{h}")
                nc.vector.reciprocal_approx_fast(rec[:], srow[:])
                bc = bcpool.tile([HD, QC], f32, tag=f"bc{h}")
                nc.gpsimd.partition_broadcast(bc[:], rec[:])
                nc.vector.tensor_tensor(y[HD * h:HD * (h + 1), :], yu[:],
                                        bc[:], op=AluOpType.mult)

            def emit_proj(a, qc, mts):
                y = y_tiles[a, qc]
                for mt in mts:
                    pso = psO.tile([128, QC], f32, tag="pso")
                    nc.tensor.matmul(
                        pso[:],
                        lhsT=wp_t[:, 128 * mt:128 * (mt + 1)],
                        rhs=y[:],
                        start=True, stop=True,
                    )
                    ot = otpool.tile([128, QC], bf16, tag="ot")
                    nc.vector.tensor_copy(ot[:], pso[:])
                    nc.sync.dma_start(
                        out[a, 128 * mt:128 * (mt + 1), QC * qc:QC * (qc + 1)],
                        ot[:],
                    )

            def kb_order(qc):
                nkb = 4 * (qc + 1)
                return ([kb for kb in range(nkb) if kb < 4 * qc] +
                        [kb for kb in range(nkb) if kb >= 4 * qc])

            def attn_units(a):
                # ordered attention stream for batch a; yields callables.
                units = []

                def slot_u(qc, kb):
                    units.append(lambda: emit_slot(a, qc, kb))

                psy_tiles = {}

                def pv_u(qc, h, kbs):
                    def f():
                        if (qc, h) not in psy_tiles:
                            psy_tiles[qc, h] = psY.tile([HD + 1, QC], f32,
                                                        tag="psy", name="psy")
                        emit_pv(a, qc, h, kbs, psy_tiles[qc, h], 4 * (qc + 1))
                    units.append(f)

                def drain_u(qc, h):
                    units.append(lambda: emit_drain(a, qc, h, psy_tiles[qc, h]))

                def proj_u(qc, mts):
                    units.append(lambda: emit_proj(a, qc, list(mts)))

                # scores for chunks 0 and 1 feed ACT early (these land
                # interleaved into QKV(a+1) via merge())
                for kb in range(4):
                    slot_u(0, kb)
                for kb in range(8):
                    slot_u(1, kb)
                ko0, ko1 = kb_order(0), kb_order(1)
                ko2, ko3 = kb_order(2), kb_order(3)
                pv_u(0, 0, ko0[:2]); slot_u(2, 0)
                pv_u(0, 0, ko0[2:]); slot_u(2, 1); drain_u(0, 0)
                pv_u(0, 1, ko0[:2]); slot_u(2, 2)
                pv_u(0, 1, ko0[2:]); slot_u(2, 3); drain_u(0, 1)
                slot_u(2, 4); pv_u(1, 0, ko1[:3])
                slot_u(2, 5); pv_u(1, 0, ko1[3:6])
                slot_u(2, 6); pv_u(1, 0, ko1[6:]); drain_u(1, 0)
                slot_u(2, 7); pv_u(1, 1, ko1[:3])
                slot_u(2, 8); pv_u(1, 1, ko1[3:6])
                slot_u(2, 9); pv_u(1, 1, ko1[6:]); drain_u(1, 1)
                slot_u(2, 10); proj_u(0, [0, 1])
                slot_u(2, 11); proj_u(0, [2, 3])
                slot_u(3, 0); proj_u(0, [4, 5])
                slot_u(3, 1); proj_u(0, [6, 7])
                slot_u(3, 2); pv_u(2, 0, ko2[:3])
                slot_u(3, 3); pv_u(2, 0, ko2[3:6])
                slot_u(3, 4); pv_u(2, 0, ko2[6:9])
                slot_u(3, 5); pv_u(2, 0, ko2[9:]); drain_u(2, 0)
                slot_u(3, 6); pv_u(2, 1, ko2[:3])
                slot_u(3, 7); pv_u(2, 1, ko2[3:6])
                slot_u(3, 8); pv_u(2, 1, ko2[6:9])
                slot_u(3, 9); pv_u(2, 1, ko2[9:]); drain_u(2, 1)
                slot_u(3, 10); proj_u(1, [0, 1])
                slot_u(3, 11); proj_u(1, [2, 3])
                slot_u(3, 12); proj_u(1, [4, 5])
                slot_u(3, 13); proj_u(1, [6, 7])
                slot_u(3, 14); pv_u(3, 0, ko3[:3])
                slot_u(3, 15); pv_u(3, 0, ko3[3:6])
                pv_u(3, 0, ko3[6:9]); proj_u(2, [0, 1])
                pv_u(3, 0, ko3[9:12]); proj_u(2, [2, 3])
                pv_u(3, 0, ko3[12:]); drain_u(3, 0)
                pv_u(3, 1, ko3[:3]); proj_u(2, [4, 5])
                pv_u(3, 1, ko3[3:6]); proj_u(2, [6, 7])
                pv_u(3, 1, ko3[6:9])
                pv_u(3, 1, ko3[9:12])
                pv_u(3, 1, ko3[12:]); drain_u(3, 1)
                # proj(3) deferred to the next section
                return units

            def qkv_units(b):
                units = []
                for c in range(NQC):
                    units.append(lambda c=c: emit_qkv_part(b, c, 0))
                    units.append(lambda c=c: emit_qkv_part(b, c, 1))
                return units

            def merge(fill, attn):
                # interleave: lead with 2 fillers, then spread the rest
                # evenly through the attention stream.
                seq = []
                lead = fill[:2]
                rest = fill[2:]
                seq += lead
                if not attn:
                    return seq + rest
                if not rest:
                    return seq + attn
                stride = max(1, len(attn) // len(rest))
                ai = 0
                for i, f in enumerate(rest):
                    nxt = min(len(attn), (i + 1) * stride)
                    seq += attn[ai:nxt]
                    seq.append(f)
                    ai = nxt
                seq += attn[ai:]
                return seq

            # ---- sections ----
            prefetch_x(0)
            prefetch_x(1)
            for b in range(B + 1):
                if b >= 2 and b < B + 1:
                    pass
                fill = []
                if b < B:
                    fill += qkv_units(b)
                if b >= 2:
                    a2 = b - 2
                    fill.append(lambda a2=a2: emit_proj(a2, 3, [0, 1, 2, 3]))
                    fill.append(lambda a2=a2: emit_proj(a2, 3, [4, 5, 6, 7]))
                attn = attn_units(b - 1) if b >= 1 else []
                for u in merge(fill, attn):
                    u()
                if b + 2 <= B - 1:
                    prefetch_x(b + 2)
            # tail: proj(B-1, 3)
            emit_proj(B - 1, 3, list(range(8)))

    nc.compile()
    return nc


def _get_nc():
    if "nc" not in _cache:
        _cache["nc"] = _build()
    return _cache["nc"]


def _make_masks():
    i = np.arange(KB)[:, None]
    j = np.arange(QC)[None, :]
    m = np.zeros((4, KB, QC), dtype=np.float32)
    for p in range(4):
        m[p] = (j >= (KB * p + i)).astype(np.float32)
    return m.astype(_BF16)


def shard_inputs(x, w_qkv, w_proj):
    xt = np.ascontiguousarray(np.asarray(x, dtype=np.float32).transpose(0, 2, 1))
    xt = xt.astype(_BF16)
    w_qkv = np.asarray(w_qkv, dtype=np.float32)
    w_proj = np.asarray(w_proj, dtype=np.float32)
    masks = _make_masks()
    in_maps = []
    for c in range(N_CORES):
        qcols = slice(FPC * c, FPC * (c + 1))
        kcols = slice(D + FPC * c, D + FPC * (c + 1))
        vcols = slice(2 * D + FPC * c, 2 * D + FPC * (c + 1))
        w3_c = np.concatenate(
            [w_qkv[:, qcols], w_qkv[:, kcols], w_qkv[:, vcols]], axis=1)
        in_maps.append({
            "xt": xt,
            "w3": np.ascontiguousarray(w3_c).astype(_BF16),
            "wp": np.ascontiguousarray(w_proj[FPC * c:FPC * (c + 1), :]).astype(_BF16),
            "masks": masks,
        })
    return in_maps


def unshard(results):
    total = results[0]["out"].astype(np.float32)
    for r in results[1:]:
        total += r["out"].astype(np.float32)
    return np.ascontiguousarray(total.transpose(0, 2, 1))


def run(inputs, trace=False, **kw):
    from concourse.bass_utils import run_bass_kernel_spmd

    nc = _get_nc()
    in_maps = shard_inputs(inputs["x"], inputs["w_qkv"], inputs["w_proj"])
    res = run_bass_kernel_spmd(nc, in_maps, core_ids=list(range(N_CORES)),
                               trace=trace, **kw)
    return unshard(res.results), res


def kernel(**inputs):
    out, _ = run(inputs, trace=False)
    return out


# revision 13
# speedup vs baseline: 1.6552x; 1.0101x over previous
"""Multi-head causal attention (B=4, T=2048, D=1024, H=16) on 8 TRN2 cores.

Tensor-parallel over heads: core c computes heads {2c, 2c+1}. Redesign vs
the previous version:
  - Score matmuls for h0 (PE rows 0-63) and h1 (rows 64-127) are emitted
    back-to-back into separate PSUM banks so the row-tiled pairs execute
    concurrently (~2x score throughput).
  - V is computed directly token-major (lhsT = x^T token slice), so there
    are no PE transposes and no transpose->copy->memset chain.
  - vaug ones-columns are persistent tiles memset once at startup (the old
    per-batch gpsimd memsets stalled PV ~30us/batch behind the norm chain).
  - Norm chain: DVE copy of the denominator row, reciprocal_approx_fast,
    gpsimd partition_broadcast, DVE multiply into y (bf16).
  - Proj drains on DVE as bf16; output DMA'd bf16 and summed on host.
  - Emission is software-pipelined: section b emits QKV(b) interleaved with
    attention of batch b-1 (scores qc0/1 during QKV, PV/proj later), and
    proj(b-1, qc3) is deferred into section b+1.
"""

import sys

for _p in ("/opt/trn_rl_repo",):
    if _p not in sys.path:
        sys.path.append(_p)

import numpy as np
import ml_dtypes

B, T, D = 4, 2048, 1024
H = 16
HD = D // H
NORM = float(np.sqrt(D))
N_CORES = 8
HEADS_PER_CORE = H // N_CORES          # 2
FPC = HEADS_PER_CORE * HD              # 128 features per core
QC = 512                               # query chunk
NQC = T // QC                          # 4
KB = 128                               # key block
DKC = D // 128                         # 8 contraction chunks over D
NTB = T // 128                         # 16 token blocks

_BF16 = ml_dtypes.bfloat16

_cache = {}

N_WARM = 140


def _build():
    import concourse.bacc as bacc
    import concourse.mybir as mybir
    from concourse.tile import TileContext
    from concourse.alu_op_type import AluOpType

    f32 = mybir.dt.float32
    bf16 = mybir.dt.bfloat16
    EXP = mybir.ActivationFunctionType.Exp

    nc = bacc.Bacc("TRN2", target_bir_lowering=False, debug=False,
                   num_devices=N_CORES)

    xt = nc.dram_tensor("xt", [B, D, T], bf16, kind="ExternalInput").ap()
    w3 = nc.dram_tensor("w3", [D, 3 * FPC], bf16, kind="ExternalInput").ap()
    wp = nc.dram_tensor("wp", [FPC, D], bf16, kind="ExternalInput").ap()
    masks = nc.dram_tensor("masks", [4, KB, QC], bf16, kind="ExternalInput").ap()
    out = nc.dram_tensor("out", [B, D, T], bf16, kind="ExternalOutput").ap()

    with TileContext(nc) as tc:
        with (
            tc.tile_pool(name="const", bufs=1) as cpool,
            tc.tile_pool(name="xp", bufs=12) as xpool,
            tc.tile_pool(name="qk", bufs=2) as qkpool,
            tc.tile_pool(name="pt", bufs=60) as ptpool,
            tc.tile_pool(name="y", bufs=6) as ypool,
            tc.tile_pool(name="sm", bufs=3) as smpool,
            tc.tile_pool(name="bcp", bufs=2) as bcpool,
            tc.tile_pool(name="ot", bufs=4) as otpool,
            tc.tile_pool(name="psA", bufs=4, space="PSUM") as psA,
            tc.tile_pool(name="psY", bufs=2, space="PSUM") as psY,
            tc.tile_pool(name="psO", bufs=2, space="PSUM") as psO,
        ):
            # ---- constants ----
            w3_t = []
            for kc in range(DKC):
                t = cpool.tile([128, 3 * FPC], bf16, tag=f"w3{kc}")
                nc.sync.dma_start(t[:], w3[kc * 128:(kc + 1) * 128, :])
                w3_t.append(t)
            wp_t = cpool.tile([FPC, D], bf16, tag="wp")
            nc.sync.dma_start(wp_t[:], wp[:])
            mask_t = []
            for p in range(4):
                t = cpool.tile([KB, QC], bf16, tag=f"mask{p}")
                nc.sync.dma_start(t[:], masks[p])
                mask_t.append(t)

            # persistent vaug tiles: [v_h0 64 | ones | v_h1 64 | ones],
            # double-buffered across batches; ones written once here.
            vaug = [[], []]
            for g in range(2):
                for tb in range(NTB):
                    va = cpool.tile([128, 2 * HD + 2], bf16, tag=f"va{g}_{tb}")
                    nc.gpsimd.memset(va[:, HD:HD + 1], 1.0)
                    nc.gpsimd.memset(va[:, 2 * HD + 1:2 * HD + 2], 1.0)
                    vaug[g].append(va)

            # PE warmup on a memset tile (no DMA dependency): keeps the HAM
            # clock-gate busy during the initial x DMA.
            wt = cpool.tile([128, 128], bf16, tag="warm")
            nc.vector.memset(wt[:], 0.25)
            psw = psO.tile([128, QC], f32, tag="pso")
            for _ in range(N_WARM):
                nc.tensor.matmul(psw[:, 0:128], lhsT=wt[:], rhs=wt[:],
                                 start=True, stop=True)

            # ---- mutable cross-section state ----
            xp_t = {}      # b -> [8 tiles]
            qkp = {}       # b -> (qp, kp)
            pts = {}       # (a, qc, h, kb) -> (pt tile, j0)
            y_tiles = {}   # (a, qc) -> y tile

            def prefetch_x(b):
                ts = []
                for kc in range(DKC):
                    t = xpool.tile([128, T], bf16, tag="xp")
                    nc.sync.dma_start(t[:], xt[b, kc * 128:(kc + 1) * 128, :])
                    ts.append(t)
                xp_t[b] = ts

            # ---- QKV units ----
            # Each chunk c is emitted as two interleaved parts so the short
            # N=128 token-major V matmuls hide their weight loads under the
            # long N=512 Q/K streams:
            #   part 0: Q[kc] + V(tb 4c+0)[kc] + V(tb 4c+1)[kc]  for kc=0..7
            #   part 1: K[kc] + V(tb 4c+2)[kc] + V(tb 4c+3)[kc]
            def emit_qk_part(b, c, part):
                if b not in qkp:
                    qp = qkpool.tile([128, T], bf16, tag="qp")
                    kp = qkpool.tile([128, T], bf16, tag="kp")
                    qkp[b] = (qp, kp)
                dst = qkp[b][part]
                ps = psO.tile([128, QC], f32, tag="pso", name="psqk")
                for kc in range(DKC):
                    nc.tensor.matmul(
                        ps[:],
                        lhsT=w3_t[kc][:, 128 * part:128 * (part + 1)],
                        rhs=xp_t[b][kc][:, QC * c:QC * (c + 1)],
                        start=(kc == 0), stop=(kc == DKC - 1),
                    )
                nc.vector.tensor_copy(dst[:, QC * c:QC * (c + 1)], ps[:])

            def emit_v_part(b, c, part):
                psv = psO.tile([128, QC], f32, tag="pso", name="psv")
                tbs = (4 * c + 2 * part, 4 * c + 2 * part + 1)
                for j, tb in enumerate(tbs):
                    for kc in range(DKC):
                        nc.tensor.matmul(
                            psv[:, 256 * j:256 * j + 128],
                            lhsT=xp_t[b][kc][:, 128 * tb:128 * (tb + 1)],
                            rhs=w3_t[kc][:, 256:384],
                            start=(kc == 0), stop=(kc == DKC - 1),
                        )
                for j, tb in enumerate(tbs):
                    va = vaug[b % 2][tb]
                    nc.vector.tensor_copy(va[:, 0:HD],
                                          psv[:, 256 * j:256 * j + HD])
                    nc.vector.tensor_copy(va[:, HD + 1:2 * HD + 1],
                                          psv[:, 256 * j + HD:256 * j + 128])

            # ---- attention units ----
            def emit_slot(a, qc, kb):
                # one key block, both heads: two concurrent row-tiled MMs
                # into separate PSUM banks, exp on ACT, mask on DVE.
                j0 = max(0, KB * (kb - 4 * qc))
                p = kb - 4 * qc
                qp, kp = qkp[a]
                ps_h = []
                for h in range(2):
                    ps = psA.tile([128, QC], f32, tag="ps")
                    nc.tensor.matmul(
                        ps[:, j0:QC],
                        lhsT=kp[HD * h:HD * (h + 1), KB * kb:KB * (kb + 1)],
                        rhs=qp[HD * h:HD * (h + 1), QC * qc + j0:QC * (qc + 1)],
                        start=True, stop=True,
                    )
                    ps_h.append(ps)
                for h in range(2):
                    pt = ptpool.tile([KB, QC], bf16, tag="pt")
                    nc.scalar.activation(pt[:, j0:QC], ps_h[h][:, j0:QC],
                                         EXP, scale=1.0 / NORM)
                    if p >= 0:
                        nc.vector.tensor_tensor(
                            pt[:, j0:QC], pt[:, j0:QC], mask_t[p][:, j0:QC],
                            op=AluOpType.mult,
                        )
                    pts[a, qc, h, kb] = (pt, j0)

            def emit_pv(a, qc, h, kbs, psy, nkb, state={}):
                for kb in kbs:
                    pt, j0 = pts.pop((a, qc, h, kb))
                    i = state.get((a, qc, h), 0)
                    nc.tensor.matmul(
                        psy[0:HD + 1, j0:QC],
                        lhsT=vaug[a % 2][kb][:, (HD + 1) * h:(HD + 1) * (h + 1)],
                        rhs=pt[:, j0:QC],
                        start=(i == 0), stop=(i == nkb - 1),
                    )
                    state[a, qc, h] = i + 1

            def emit_drain(a, qc, h, psy):
                if (a, qc) not in y_tiles:
                    y_tiles[a, qc] = ypool.tile([FPC, QC], bf16, tag="y", name="y")
                y = y_tiles[a, qc]
                srow = smpool.tile([1, QC], f32, tag=f"srow{h}")
                nc.vector.tensor_copy(srow[:], psy[HD:HD + 1, :])
                yu = smpool.tile([HD, QC], f32, tag=f"yu{h}")
                nc.vector.tensor_copy(yu[:], psy[0:HD, :])
                rec = smpool.tile([1, QC], f32, tag=f"rec{h}")
                nc.vector.reciprocal_approx_fast(rec[:], srow[:])
                bc = bcpool.tile([HD, QC], f32, tag=f"bc{h}")
                nc.gpsimd.partition_broadcast(bc[:], rec[:])
                nc.vector.tensor_tensor(y[HD * h:HD * (h + 1), :], yu[:],
                                        bc[:], op=AluOpType.mult)

            def emit_proj(a, qc, mts):
                y = y_tiles[a, qc]
                for mt in mts:
                    pso = psO.tile([128, QC], f32, tag="pso")
                    nc.tensor.matmul(
                        pso[:],
                        lhsT=wp_t[:, 128 * mt:128 * (mt + 1)],
                        rhs=y[:],
                        start=True, stop=True,
                    )
                    ot = otpool.tile([128, QC], bf16, tag="ot")
                    nc.vector.tensor_copy(ot[:], pso[:])
                    nc.sync.dma_start(
                        out[a, 128 * mt:128 * (mt + 1), QC * qc:QC * (qc + 1)],
                        ot[:],
                    )

            def kb_order(qc):
                nkb = 4 * (qc + 1)
                return ([kb for kb in range(nkb) if kb < 4 * qc] +
                        [kb for kb in range(nkb) if kb >= 4 * qc])

            def attn_units(a):
                # ordered attention stream for batch a; yields callables.
                units = []

                def slot_u(qc, kb):
                    units.append(lambda: emit_slot(a, qc, kb))

                psy_tiles = {}

                def pv_u(qc, h, kbs):
                    def f():
                        if (qc, h) not in psy_tiles:
                            psy_tiles[qc, h] = psY.tile([HD + 1, QC], f32,
                                                        tag="psy", name="psy")
                        emit_pv(a, qc, h, kbs, psy_tiles[qc, h], 4 * (qc + 1))
                    units.append(f)

                def drain_u(qc, h):
                    units.append(lambda: emit_drain(a, qc, h, psy_tiles[qc, h]))

                def proj_u(qc, mts):
                    units.append(lambda: emit_proj(a, qc, list(mts)))

                # scores for chunks 0 and 1 feed ACT early (these land
                # interleaved into QKV(a+1) via merge())
                for kb in range(4):
                    slot_u(0, kb)
                for kb in range(8):
                    slot_u(1, kb)
                ko0, ko1 = kb_order(0), kb_order(1)
                ko2, ko3 = kb_order(2), kb_order(3)
                pv_u(0, 0, ko0[:2]); slot_u(2, 0)
                pv_u(0, 1, ko0[:2]); slot_u(2, 1)
                pv_u(0, 0, ko0[2:]); drain_u(0, 0); slot_u(2, 2)
                pv_u(0, 1, ko0[2:]); drain_u(0, 1); slot_u(2, 3)
                slot_u(2, 4); pv_u(1, 0, ko1[:3])
                slot_u(2, 5); pv_u(1, 1, ko1[:3])
                slot_u(2, 6); pv_u(1, 0, ko1[3:6])
                slot_u(2, 7); pv_u(1, 1, ko1[3:6])
                slot_u(2, 8); pv_u(1, 0, ko1[6:]); drain_u(1, 0)
                slot_u(2, 9); pv_u(1, 1, ko1[6:]); drain_u(1, 1)
                slot_u(2, 10); proj_u(0, [0, 1])
                slot_u(2, 11); proj_u(0, [2, 3])
                slot_u(3, 0); proj_u(0, [4, 5])
                slot_u(3, 1); proj_u(0, [6, 7])
                slot_u(3, 2); pv_u(2, 0, ko2[:3])
                slot_u(3, 3); pv_u(2, 1, ko2[:3])
                slot_u(3, 4); pv_u(2, 0, ko2[3:6])
                slot_u(3, 5); pv_u(2, 1, ko2[3:6])
                slot_u(3, 6); pv_u(2, 0, ko2[6:9])
                slot_u(3, 7); pv_u(2, 1, ko2[6:9])
                slot_u(3, 8); pv_u(2, 0, ko2[9:]); drain_u(2, 0)
                slot_u(3, 9); pv_u(2, 1, ko2[9:]); drain_u(2, 1)
                slot_u(3, 10); proj_u(1, [0, 1])
                slot_u(3, 11); proj_u(1, [2, 3])
                slot_u(3, 12); proj_u(1, [4, 5])
                slot_u(3, 13); proj_u(1, [6, 7])
                slot_u(3, 14); pv_u(3, 0, ko3[:3])
                slot_u(3, 15); pv_u(3, 1, ko3[:3])
                pv_u(3, 0, ko3[3:6]); proj_u(2, [0, 1])
                pv_u(3, 1, ko3[3:6]); proj_u(2, [2, 3])
                pv_u(3, 0, ko3[6:9]); proj_u(2, [4, 5])
                pv_u(3, 1, ko3[6:9]); proj_u(2, [6, 7])
                pv_u(3, 0, ko3[9:12])
                pv_u(3, 1, ko3[9:12])
                pv_u(3, 0, ko3[12:]); drain_u(3, 0)
                pv_u(3, 1, ko3[12:]); drain_u(3, 1)
                # proj(3) deferred to the next section
                return units

            def qkv_units(b):
                units = []
                for c in range(NQC):
                    units.append(lambda c=c: emit_qk_part(b, c, 0))
                    units.append(lambda c=c: emit_v_part(b, c, 0))
                    units.append(lambda c=c: emit_qk_part(b, c, 1))
                    units.append(lambda c=c: emit_v_part(b, c, 1))
                return units

            def merge(fill, attn):
                # interleave: lead with 2 fillers, then spread the rest
                # evenly through the attention stream.
                seq = []
                lead = fill[:2]
                rest = fill[2:]
                seq += lead
                if not attn:
                    return seq + rest
                if not rest:
                    return seq + attn
                stride = max(1, len(attn) // len(rest))
                ai = 0
                for i, f in enumerate(rest):
                    nxt = min(len(attn), (i + 1) * stride)
                    seq += attn[ai:nxt]
                    seq.append(f)
                    ai = nxt
                seq += attn[ai:]
                return seq

            # ---- sections ----
            prefetch_x(0)
            prefetch_x(1)
            for b in range(B + 1):
                if b >= 2 and b < B + 1:
                    pass
                fill = []
                if b < B:
                    fill += qkv_units(b)
                if b >= 2:
                    a2 = b - 2
                    fill.append(lambda a2=a2: emit_proj(a2, 3, [0, 1, 2, 3]))
                    fill.append(lambda a2=a2: emit_proj(a2, 3, [4, 5, 6, 7]))
                attn = attn_units(b - 1) if b >= 1 else []
                for u in merge(fill, attn):
                    u()
                if b + 2 <= B - 1:
                    prefetch_x(b + 2)
            # tail: proj(B-1, 3)
            emit_proj(B - 1, 3, list(range(8)))

    nc.compile()
    return nc


def _get_nc():
    if "nc" not in _cache:
        _cache["nc"] = _build()
    return _cache["nc"]


def _make_masks():
    i = np.arange(KB)[:, None]
    j = np.arange(QC)[None, :]
    m = np.zeros((4, KB, QC), dtype=np.float32)
    for p in range(4):
        m[p] = (j >= (KB * p + i)).astype(np.float32)
    return m.astype(_BF16)


def shard_inputs(x, w_qkv, w_proj):
    xt = np.ascontiguousarray(np.asarray(x, dtype=np.float32).transpose(0, 2, 1))
    xt = xt.astype(_BF16)
    w_qkv = np.asarray(w_qkv, dtype=np.float32)
    w_proj = np.asarray(w_proj, dtype=np.float32)
    masks = _make_masks()
    in_maps = []
    for c in range(N_CORES):
        qcols = slice(FPC * c, FPC * (c + 1))
        kcols = slice(D + FPC * c, D + FPC * (c + 1))
        vcols = slice(2 * D + FPC * c, 2 * D + FPC * (c + 1))
        w3_c = np.concatenate(
            [w_qkv[:, qcols], w_qkv[:, kcols], w_qkv[:, vcols]], axis=1)
        in_maps.append({
            "xt": xt,
            "w3": np.ascontiguousarray(w3_c).astype(_BF16),
            "wp": np.ascontiguousarray(w_proj[FPC * c:FPC * (c + 1), :]).astype(_BF16),
            "masks": masks,
        })
    return in_maps


def unshard(results):
    total = results[0]["out"].astype(np.float32)
    for r in results[1:]:
        total += r["out"].astype(np.float32)
    return np.ascontiguousarray(total.transpose(0, 2, 1))


def run(inputs, trace=False, **kw):
    from concourse.bass_utils import run_bass_kernel_spmd

    nc = _get_nc()
    in_maps = shard_inputs(inputs["x"], inputs["w_qkv"], inputs["w_proj"])
    res = run_bass_kernel_spmd(nc, in_maps, core_ids=list(range(N_CORES)),
                               trace=trace, **kw)
    return unshard(res.results), res


def kernel(**inputs):
    out, _ = run(inputs, trace=False)
    return out


# revision 14
# speedup vs baseline: 1.7132x; 1.0351x over previous
"""Multi-head causal attention (B=4, T=2048, D=1024, H=16) on 8 TRN2 cores.

Tensor-parallel over heads: core c computes heads {2c, 2c+1}. Redesign vs
the previous version:
  - Score matmuls for h0 (PE rows 0-63) and h1 (rows 64-127) are emitted
    back-to-back into separate PSUM banks so the row-tiled pairs execute
    concurrently (~2x score throughput).
  - V is computed directly token-major (lhsT = x^T token slice), so there
    are no PE transposes and no transpose->copy->memset chain.
  - vaug ones-columns are persistent tiles memset once at startup (the old
    per-batch gpsimd memsets stalled PV ~30us/batch behind the norm chain).
  - Norm chain: DVE copy of the denominator row, reciprocal_approx_fast,
    gpsimd partition_broadcast, DVE multiply into y (bf16).
  - Proj drains on DVE as bf16; output DMA'd bf16 and summed on host.
  - Emission is software-pipelined: section b emits QKV(b) interleaved with
    attention of batch b-1 (scores qc0/1 during QKV, PV/proj later), and
    proj(b-1, qc3) is deferred into section b+1.
"""

import sys

for _p in ("/opt/trn_rl_repo",):
    if _p not in sys.path:
        sys.path.append(_p)

import numpy as np
import ml_dtypes

B, T, D = 4, 2048, 1024
H = 16
HD = D // H
NORM = float(np.sqrt(D))
N_CORES = 8
HEADS_PER_CORE = H // N_CORES          # 2
FPC = HEADS_PER_CORE * HD              # 128 features per core
QC = 512                               # query chunk
NQC = T // QC                          # 4
KB = 128                               # key block
DKC = D // 128                         # 8 contraction chunks over D
NTB = T // 128                         # 16 token blocks

_BF16 = ml_dtypes.bfloat16

_cache = {}

N_WARM = 140


def _build():
    import concourse.bacc as bacc
    import concourse.mybir as mybir
    from concourse.tile import TileContext
    from concourse.alu_op_type import AluOpType

    f32 = mybir.dt.float32
    bf16 = mybir.dt.bfloat16
    EXP = mybir.ActivationFunctionType.Exp

    nc = bacc.Bacc("TRN2", target_bir_lowering=False, debug=False,
                   num_devices=N_CORES)

    xt = nc.dram_tensor("xt", [B, D, T], bf16, kind="ExternalInput").ap()
    w3 = nc.dram_tensor("w3", [D, 3 * FPC], bf16, kind="ExternalInput").ap()
    wp = nc.dram_tensor("wp", [FPC, D], bf16, kind="ExternalInput").ap()
    masks = nc.dram_tensor("masks", [4, KB, QC], bf16, kind="ExternalInput").ap()
    out = nc.dram_tensor("out", [B, D, T], bf16, kind="ExternalOutput").ap()

    with TileContext(nc) as tc:
        with (
            tc.tile_pool(name="const", bufs=1) as cpool,
            tc.tile_pool(name="xp", bufs=12) as xpool,
            tc.tile_pool(name="qk", bufs=2) as qkpool,
            tc.tile_pool(name="pt", bufs=60) as ptpool,
            tc.tile_pool(name="y", bufs=6) as ypool,
            tc.tile_pool(name="sm", bufs=3) as smpool,
            tc.tile_pool(name="bcp", bufs=2) as bcpool,
            tc.tile_pool(name="ot", bufs=4) as otpool,
            tc.tile_pool(name="psA", bufs=4, space="PSUM") as psA,
            tc.tile_pool(name="psY", bufs=2, space="PSUM") as psY,
            tc.tile_pool(name="psO", bufs=2, space="PSUM") as psO,
        ):
            # ---- constants ----
            w3_t = []
            for kc in range(DKC):
                t = cpool.tile([128, 3 * FPC], bf16, tag=f"w3{kc}")
                nc.sync.dma_start(t[:], w3[kc * 128:(kc + 1) * 128, :])
                w3_t.append(t)
            wp_t = cpool.tile([FPC, D], bf16, tag="wp")
            nc.sync.dma_start(wp_t[:], wp[:])
            mask_t = []
            for p in range(4):
                t = cpool.tile([KB, QC], bf16, tag=f"mask{p}")
                nc.sync.dma_start(t[:], masks[p])
                mask_t.append(t)

            # persistent vaug tiles: [v_h0 64 | ones | v_h1 64 | ones],
            # double-buffered across batches; ones written once here.
            vaug = [[], []]
            for g in range(2):
                for tb in range(NTB):
                    va = cpool.tile([128, 2 * HD + 2], bf16, tag=f"va{g}_{tb}")
                    nc.gpsimd.memset(va[:, HD:HD + 1], 1.0)
                    nc.gpsimd.memset(va[:, 2 * HD + 1:2 * HD + 2], 1.0)
                    vaug[g].append(va)

            # PE warmup on a memset tile (no DMA dependency): keeps the HAM
            # clock-gate busy during the initial x DMA.
            wt = cpool.tile([128, 128], bf16, tag="warm")
            nc.vector.memset(wt[:], 0.25)
            psw = psO.tile([128, QC], f32, tag="pso")
            for _ in range(N_WARM):
                nc.tensor.matmul(psw[:, 0:128], lhsT=wt[:], rhs=wt[:],
                                 start=True, stop=True)

            # ---- mutable cross-section state ----
            xp_t = {}      # b -> [8 tiles]
            qkp = {}       # b -> (qp, kp)
            pts = {}       # (a, qc, h, kb) -> (pt tile, j0)
            y_tiles = {}   # (a, qc) -> y tile

            def prefetch_x(b):
                ts = []
                for kc in range(DKC):
                    t = xpool.tile([128, T], bf16, tag="xp")
                    nc.sync.dma_start(t[:], xt[b, kc * 128:(kc + 1) * 128, :])
                    ts.append(t)
                xp_t[b] = ts

            # ---- QKV units ----
            # Each chunk c is emitted as two interleaved parts so the short
            # N=128 token-major V matmuls hide their weight loads under the
            # long N=512 Q/K streams:
            #   part 0: Q[kc] + V(tb 4c+0)[kc] + V(tb 4c+1)[kc]  for kc=0..7
            #   part 1: K[kc] + V(tb 4c+2)[kc] + V(tb 4c+3)[kc]
            def emit_qk_part(b, c, part):
                if b not in qkp:
                    qp = qkpool.tile([128, T], bf16, tag="qp")
                    kp = qkpool.tile([128, T], bf16, tag="kp")
                    qkp[b] = (qp, kp)
                dst = qkp[b][part]
                ps = psO.tile([128, QC], f32, tag="pso", name="psqk")
                for kc in range(DKC):
                    nc.tensor.matmul(
                        ps[:],
                        lhsT=w3_t[kc][:, 128 * part:128 * (part + 1)],
                        rhs=xp_t[b][kc][:, QC * c:QC * (c + 1)],
                        start=(kc == 0), stop=(kc == DKC - 1),
                    )
                nc.vector.tensor_copy(dst[:, QC * c:QC * (c + 1)], ps[:])

            def emit_v_part(b, c, part):
                psv = psO.tile([128, QC], f32, tag="pso", name="psv")
                tbs = (4 * c + 2 * part, 4 * c + 2 * part + 1)
                for j, tb in enumerate(tbs):
                    for kc in range(DKC):
                        nc.tensor.matmul(
                            psv[:, 256 * j:256 * j + 128],
                            lhsT=xp_t[b][kc][:, 128 * tb:128 * (tb + 1)],
                            rhs=w3_t[kc][:, 256:384],
                            start=(kc == 0), stop=(kc == DKC - 1),
                        )
                for j, tb in enumerate(tbs):
                    va = vaug[b % 2][tb]
                    nc.vector.tensor_copy(va[:, 0:HD],
                                          psv[:, 256 * j:256 * j + HD])
                    nc.vector.tensor_copy(va[:, HD + 1:2 * HD + 1],
                                          psv[:, 256 * j + HD:256 * j + 128])

            # ---- attention units ----
            def emit_slot(a, qc, kb):
                # one key block, both heads: two concurrent row-tiled MMs
                # into separate PSUM banks, exp on ACT, mask on DVE.
                j0 = max(0, KB * (kb - 4 * qc))
                p = kb - 4 * qc
                qp, kp = qkp[a]
                ps_h = []
                for h in range(2):
                    ps = psA.tile([128, QC], f32, tag="ps")
                    nc.tensor.matmul(
                        ps[:, j0:QC],
                        lhsT=kp[HD * h:HD * (h + 1), KB * kb:KB * (kb + 1)],
                        rhs=qp[HD * h:HD * (h + 1), QC * qc + j0:QC * (qc + 1)],
                        start=True, stop=True,
                    )
                    ps_h.append(ps)
                for h in range(2):
                    pt = ptpool.tile([KB, QC], bf16, tag="pt")
                    nc.scalar.activation(pt[:, j0:QC], ps_h[h][:, j0:QC],
                                         EXP, scale=1.0 / NORM)
                    if p >= 0:
                        # only the leading 128 columns of a diagonal block
                        # straddle the causal boundary; the rest is unmasked
                        nc.vector.tensor_tensor(
                            pt[:, j0:j0 + KB], pt[:, j0:j0 + KB],
                            mask_t[0][:, 0:KB],
                            op=AluOpType.mult,
                        )
                    pts[a, qc, h, kb] = (pt, j0)

            def emit_pv(a, qc, h, kbs, psy, nkb, state={}):
                for kb in kbs:
                    pt, j0 = pts.pop((a, qc, h, kb))
                    i = state.get((a, qc, h), 0)
                    nc.tensor.matmul(
                        psy[0:HD + 1, j0:QC],
                        lhsT=vaug[a % 2][kb][:, (HD + 1) * h:(HD + 1) * (h + 1)],
                        rhs=pt[:, j0:QC],
                        start=(i == 0), stop=(i == nkb - 1),
                    )
                    state[a, qc, h] = i + 1

            def emit_drain(a, qc, h, psy):
                if (a, qc) not in y_tiles:
                    y_tiles[a, qc] = ypool.tile([FPC, QC], bf16, tag="y", name="y")
                y = y_tiles[a, qc]
                srow = smpool.tile([1, QC], f32, tag=f"srow{h}")
                nc.vector.tensor_copy(srow[:], psy[HD:HD + 1, :])
                yu = smpool.tile([HD, QC], f32, tag=f"yu{h}")
                nc.vector.tensor_copy(yu[:], psy[0:HD, :])
                rec = smpool.tile([1, QC], f32, tag=f"rec{h}")
                nc.vector.reciprocal_approx_fast(rec[:], srow[:])
                bc = bcpool.tile([HD, QC], f32, tag=f"bc{h}")
                nc.gpsimd.partition_broadcast(bc[:], rec[:])
                nc.vector.tensor_tensor(y[HD * h:HD * (h + 1), :], yu[:],
                                        bc[:], op=AluOpType.mult)

            def emit_proj(a, qc, mts):
                y = y_tiles[a, qc]
                for mt in mts:
                    pso = psO.tile([128, QC], f32, tag="pso")
                    nc.tensor.matmul(
                        pso[:],
                        lhsT=wp_t[:, 128 * mt:128 * (mt + 1)],
                        rhs=y[:],
                        start=True, stop=True,
                    )
                    ot = otpool.tile([128, QC], bf16, tag="ot")
                    nc.vector.tensor_copy(ot[:], pso[:])
                    nc.sync.dma_start(
                        out[a, 128 * mt:128 * (mt + 1), QC * qc:QC * (qc + 1)],
                        ot[:],
                    )

            def kb_order(qc):
                nkb = 4 * (qc + 1)
                return ([kb for kb in range(nkb) if kb < 4 * qc] +
                        [kb for kb in range(nkb) if kb >= 4 * qc])

            def attn_units(a):
                # ordered attention stream for batch a; yields callables.
                units = []

                def slot_u(qc, kb):
                    units.append(lambda: emit_slot(a, qc, kb))

                psy_tiles = {}

                def pv_u(qc, h, kbs):
                    def f():
                        if (qc, h) not in psy_tiles:
                            psy_tiles[qc, h] = psY.tile([HD + 1, QC], f32,
                                                        tag="psy", name="psy")
                        emit_pv(a, qc, h, kbs, psy_tiles[qc, h], 4 * (qc + 1))
                    units.append(f)

                def drain_u(qc, h):
                    units.append(lambda: emit_drain(a, qc, h, psy_tiles[qc, h]))

                def proj_u(qc, mts):
                    units.append(lambda: emit_proj(a, qc, list(mts)))

                # scores for chunks 0 and 1 feed ACT early (these land
                # interleaved into QKV(a+1) via merge())
                for kb in range(4):
                    slot_u(0, kb)
                for kb in range(8):
                    slot_u(1, kb)
                ko0, ko1 = kb_order(0), kb_order(1)
                ko2, ko3 = kb_order(2), kb_order(3)
                pv_u(0, 0, ko0[:2]); slot_u(2, 0)
                pv_u(0, 1, ko0[:2]); slot_u(2, 1)
                pv_u(0, 0, ko0[2:]); drain_u(0, 0); slot_u(2, 2)
                pv_u(0, 1, ko0[2:]); drain_u(0, 1); slot_u(2, 3)
                slot_u(2, 4); pv_u(1, 0, ko1[:3])
                slot_u(2, 5); pv_u(1, 1, ko1[:3])
                slot_u(2, 6); pv_u(1, 0, ko1[3:6])
                slot_u(2, 7); pv_u(1, 1, ko1[3:6])
                slot_u(2, 8); pv_u(1, 0, ko1[6:]); drain_u(1, 0)
                slot_u(2, 9); pv_u(1, 1, ko1[6:]); drain_u(1, 1)
                slot_u(2, 10); proj_u(0, [0, 1])
                slot_u(2, 11); proj_u(0, [2, 3])
                slot_u(3, 0); proj_u(0, [4, 5])
                slot_u(3, 1); proj_u(0, [6, 7])
                slot_u(3, 2); pv_u(2, 0, ko2[:3])
                slot_u(3, 3); pv_u(2, 1, ko2[:3])
                slot_u(3, 4); pv_u(2, 0, ko2[3:6])
                slot_u(3, 5); pv_u(2, 1, ko2[3:6])
                slot_u(3, 6); pv_u(2, 0, ko2[6:9])
                slot_u(3, 7); pv_u(2, 1, ko2[6:9])
                slot_u(3, 8); pv_u(2, 0, ko2[9:]); drain_u(2, 0)
                slot_u(3, 9); pv_u(2, 1, ko2[9:]); drain_u(2, 1)
                slot_u(3, 10); proj_u(1, [0, 1])
                slot_u(3, 11); proj_u(1, [2, 3])
                slot_u(3, 12); proj_u(1, [4, 5])
                slot_u(3, 13); proj_u(1, [6, 7])
                slot_u(3, 14); pv_u(3, 0, ko3[:3])
                slot_u(3, 15); pv_u(3, 1, ko3[:3])
                pv_u(3, 0, ko3[3:6]); proj_u(2, [0, 1])
                pv_u(3, 1, ko3[3:6]); proj_u(2, [2, 3])
                pv_u(3, 0, ko3[6:9]); proj_u(2, [4, 5])
                pv_u(3, 1, ko3[6:9]); proj_u(2, [6, 7])
                pv_u(3, 0, ko3[9:12])
                pv_u(3, 1, ko3[9:12])
                pv_u(3, 0, ko3[12:]); drain_u(3, 0)
                pv_u(3, 1, ko3[12:]); drain_u(3, 1)
                # proj(3) deferred to the next section
                return units

            def qkv_units(b):
                units = []
                for c in range(NQC):
                    units.append(lambda c=c: emit_qk_part(b, c, 0))
                    units.append(lambda c=c: emit_v_part(b, c, 0))
                    units.append(lambda c=c: emit_qk_part(b, c, 1))
                    units.append(lambda c=c: emit_v_part(b, c, 1))
                return units

            def merge(fill, attn):
                # interleave: lead with 2 fillers, then spread the rest
                # evenly through the attention stream.
                seq = []
                lead = fill[:2]
                rest = fill[2:]
                seq += lead
                if not attn:
                    return seq + rest
                if not rest:
                    return seq + attn
                stride = max(1, len(attn) // len(rest))
                ai = 0
                for i, f in enumerate(rest):
                    nxt = min(len(attn), (i + 1) * stride)
                    seq += attn[ai:nxt]
                    seq.append(f)
                    ai = nxt
                seq += attn[ai:]
                return seq

            # ---- sections ----
            prefetch_x(0)
            prefetch_x(1)
            for b in range(B + 1):
                if b >= 2 and b < B + 1:
                    pass
                fill = []
                if b < B:
                    fill += qkv_units(b)
                if b >= 2:
                    a2 = b - 2
                    fill.append(lambda a2=a2: emit_proj(a2, 3, [0, 1, 2, 3]))
                    fill.append(lambda a2=a2: emit_proj(a2, 3, [4, 5, 6, 7]))
                attn = attn_units(b - 1) if b >= 1 else []
                for u in merge(fill, attn):
                    u()
                if b + 2 <= B - 1:
                    prefetch_x(b + 2)
            # tail: proj(B-1, 3)
            emit_proj(B - 1, 3, list(range(8)))

    nc.compile()
    return nc


def _get_nc():
    if "nc" not in _cache:
        _cache["nc"] = _build()
    return _cache["nc"]


def _make_masks():
    i = np.arange(KB)[:, None]
    j = np.arange(QC)[None, :]
    m = np.zeros((4, KB, QC), dtype=np.float32)
    for p in range(4):
        m[p] = (j >= (KB * p + i)).astype(np.float32)
    return m.astype(_BF16)


def shard_inputs(x, w_qkv, w_proj):
    xt = np.ascontiguousarray(np.asarray(x, dtype=np.float32).transpose(0, 2, 1))
    xt = xt.astype(_BF16)
    w_qkv = np.asarray(w_qkv, dtype=np.float32)
    w_proj = np.asarray(w_proj, dtype=np.float32)
    masks = _make_masks()
    in_maps = []
    for c in range(N_CORES):
        qcols = slice(FPC * c, FPC * (c + 1))
        kcols = slice(D + FPC * c, D + FPC * (c + 1))
        vcols = slice(2 * D + FPC * c, 2 * D + FPC * (c + 1))
        w3_c = np.concatenate(
            [w_qkv[:, qcols], w_qkv[:, kcols], w_qkv[:, vcols]], axis=1)
        in_maps.append({
            "xt": xt,
            "w3": np.ascontiguousarray(w3_c).astype(_BF16),
            "wp": np.ascontiguousarray(w_proj[FPC * c:FPC * (c + 1), :]).astype(_BF16),
            "masks": masks,
        })
    return in_maps


def unshard(results):
    total = results[0]["out"].astype(np.float32)
    for r in results[1:]:
        total += r["out"].astype(np.float32)
    return np.ascontiguousarray(total.transpose(0, 2, 1))


def run(inputs, trace=False, **kw):
    from concourse.bass_utils import run_bass_kernel_spmd

    nc = _get_nc()
    in_maps = shard_inputs(inputs["x"], inputs["w_qkv"], inputs["w_proj"])
    res = run_bass_kernel_spmd(nc, in_maps, core_ids=list(range(N_CORES)),
                               trace=trace, **kw)
    return unshard(res.results), res


def kernel(**inputs):
    out, _ = run(inputs, trace=False)
    return out


# revision 17
# speedup vs baseline: 1.7851x; 1.0420x over previous
"""Multi-head causal attention (B=4, T=2048, D=1024, H=16) on 8 TRN2 cores.

Tensor-parallel over heads: core c computes heads {2c, 2c+1}. Redesign vs
the previous version:
  - Score matmuls for h0 (PE rows 0-63) and h1 (rows 64-127) are emitted
    back-to-back into separate PSUM banks so the row-tiled pairs execute
    concurrently (~2x score throughput).
  - V is computed directly token-major (lhsT = x^T token slice), so there
    are no PE transposes and no transpose->copy->memset chain.
  - vaug ones-columns are persistent tiles memset once at startup (the old
    per-batch gpsimd memsets stalled PV ~30us/batch behind the norm chain).
  - Norm chain: DVE copy of the denominator row, reciprocal_approx_fast,
    gpsimd partition_broadcast, DVE multiply into y (bf16).
  - Proj drains on DVE as bf16; output DMA'd bf16 and summed on host.
  - Emission is software-pipelined: section b emits QKV(b) interleaved with
    attention of batch b-1 (scores qc0/1 during QKV, PV/proj later), and
    proj(b-1, qc3) is deferred into section b+1.
"""

import sys

for _p in ("/opt/trn_rl_repo",):
    if _p not in sys.path:
        sys.path.append(_p)

import numpy as np
import ml_dtypes

B, T, D = 4, 2048, 1024
H = 16
HD = D // H
NORM = float(np.sqrt(D))
N_CORES = 8
HEADS_PER_CORE = H // N_CORES          # 2
FPC = HEADS_PER_CORE * HD              # 128 features per core
QC = 512                               # query chunk
NQC = T // QC                          # 4
KB = 128                               # key block
DKC = D // 128                         # 8 contraction chunks over D
NTB = T // 128                         # 16 token blocks

_BF16 = ml_dtypes.bfloat16

_cache = {}

N_WARM = 140


def _build():
    import concourse.bacc as bacc
    import concourse.mybir as mybir
    from concourse.tile import TileContext
    from concourse.alu_op_type import AluOpType

    f32 = mybir.dt.float32
    bf16 = mybir.dt.bfloat16
    EXP = mybir.ActivationFunctionType.Exp

    nc = bacc.Bacc("TRN2", target_bir_lowering=False, debug=False,
                   num_devices=N_CORES)

    xt = nc.dram_tensor("xt", [B, D, T], bf16, kind="ExternalInput").ap()
    w3 = nc.dram_tensor("w3", [D, 3 * FPC], bf16, kind="ExternalInput").ap()
    wp = nc.dram_tensor("wp", [FPC, D], bf16, kind="ExternalInput").ap()
    masks = nc.dram_tensor("masks", [4, KB, QC], bf16, kind="ExternalInput").ap()
    out = nc.dram_tensor("out", [B, D, T], bf16, kind="ExternalOutput").ap()

    with TileContext(nc) as tc:
        with (
            tc.tile_pool(name="const", bufs=1) as cpool,
            tc.tile_pool(name="xp", bufs=12) as xpool,
            tc.tile_pool(name="qk", bufs=2) as qkpool,
            tc.tile_pool(name="pt", bufs=30) as ptpool,
            tc.tile_pool(name="y", bufs=6) as ypool,
            tc.tile_pool(name="sm", bufs=3) as smpool,
            tc.tile_pool(name="bcp", bufs=2) as bcpool,
            tc.tile_pool(name="ot", bufs=4) as otpool,
            tc.tile_pool(name="psA", bufs=2, space="PSUM") as psA,
            tc.tile_pool(name="psY", bufs=2, space="PSUM") as psY,
            tc.tile_pool(name="psO", bufs=2, space="PSUM") as psO,
        ):
            # ---- constants ----
            w3_t = []
            for kc in range(DKC):
                t = cpool.tile([128, 3 * FPC], bf16, tag=f"w3{kc}")
                nc.sync.dma_start(t[:], w3[kc * 128:(kc + 1) * 128, :])
                w3_t.append(t)
            wp_t = cpool.tile([FPC, D], bf16, tag="wp")
            nc.sync.dma_start(wp_t[:], wp[:])
            mask_t = []
            for p in range(4):
                t = cpool.tile([KB, QC], bf16, tag=f"mask{p}")
                nc.sync.dma_start(t[:], masks[p])
                mask_t.append(t)

            # persistent vaug tiles: [v_h0 64 | ones | v_h1 64 | ones],
            # double-buffered across batches; ones written once here.
            vaug = [[], []]
            for g in range(2):
                for tb in range(NTB):
                    va = cpool.tile([128, 2 * HD + 2], bf16, tag=f"va{g}_{tb}")
                    nc.gpsimd.memset(va[:, HD:HD + 1], 1.0)
                    nc.gpsimd.memset(va[:, 2 * HD + 1:2 * HD + 2], 1.0)
                    vaug[g].append(va)

            # PE warmup on a memset tile (no DMA dependency): keeps the HAM
            # clock-gate busy during the initial x DMA.
            wt = cpool.tile([128, 128], bf16, tag="warm")
            nc.vector.memset(wt[:], 0.25)
            psw = psO.tile([128, QC], f32, tag="pso")
            for _ in range(N_WARM):
                nc.tensor.matmul(psw[:, 0:128], lhsT=wt[:], rhs=wt[:],
                                 start=True, stop=True)

            # ---- mutable cross-section state ----
            xp_t = {}      # b -> [8 tiles]
            qkp = {}       # b -> (qp, kp)
            pts = {}       # (a, qc, h, kb) -> (pt tile, j0)
            y_tiles = {}   # (a, qc) -> y tile

            def prefetch_x(b):
                ts = []
                for kc in range(DKC):
                    t = xpool.tile([128, T], bf16, tag="xp")
                    nc.sync.dma_start(t[:], xt[b, kc * 128:(kc + 1) * 128, :])
                    ts.append(t)
                xp_t[b] = ts

            # ---- QKV units ----
            # Each chunk c is emitted as two interleaved parts so the short
            # N=128 token-major V matmuls hide their weight loads under the
            # long N=512 Q/K streams:
            #   part 0: Q[kc] + V(tb 4c+0)[kc] + V(tb 4c+1)[kc]  for kc=0..7
            #   part 1: K[kc] + V(tb 4c+2)[kc] + V(tb 4c+3)[kc]
            def emit_qk_part(b, c, part):
                if b not in qkp:
                    qp = qkpool.tile([128, T], bf16, tag="qp")
                    kp = qkpool.tile([128, T], bf16, tag="kp")
                    qkp[b] = (qp, kp)
                dst = qkp[b][part]
                ps = psO.tile([128, QC], f32, tag="pso", name="psqk")
                for kc in range(DKC):
                    nc.tensor.matmul(
                        ps[:],
                        lhsT=w3_t[kc][:, 128 * part:128 * (part + 1)],
                        rhs=xp_t[b][kc][:, QC * c:QC * (c + 1)],
                        start=(kc == 0), stop=(kc == DKC - 1),
                    )
                nc.vector.tensor_copy(dst[:, QC * c:QC * (c + 1)], ps[:])

            def emit_v_part(b, c, part):
                psv = psO.tile([128, QC], f32, tag="pso", name="psv")
                tbs = (4 * c + 2 * part, 4 * c + 2 * part + 1)
                for j, tb in enumerate(tbs):
                    for kc in range(DKC):
                        nc.tensor.matmul(
                            psv[:, 256 * j:256 * j + 128],
                            lhsT=xp_t[b][kc][:, 128 * tb:128 * (tb + 1)],
                            rhs=w3_t[kc][:, 256:384],
                            start=(kc == 0), stop=(kc == DKC - 1),
                        )
                for j, tb in enumerate(tbs):
                    va = vaug[b % 2][tb]
                    nc.vector.tensor_copy(va[:, 0:HD],
                                          psv[:, 256 * j:256 * j + HD])
                    nc.vector.tensor_copy(va[:, HD + 1:2 * HD + 1],
                                          psv[:, 256 * j + HD:256 * j + 128])

            # ---- attention units ----
            def emit_slot(a, qc, kb):
                # one key block, both heads: two concurrent row-tiled MMs
                # into the two banks of one PSUM pair tile, exp on ACT,
                # triangular-boundary mask on DVE.
                j0 = max(0, KB * (kb - 4 * qc))
                p = kb - 4 * qc
                qp, kp = qkp[a]
                ps = psA.tile([128, 2 * QC], f32, tag="ps")
                for h in range(2):
                    nc.tensor.matmul(
                        ps[:, QC * h + j0:QC * (h + 1)],
                        lhsT=kp[HD * h:HD * (h + 1), KB * kb:KB * (kb + 1)],
                        rhs=qp[HD * h:HD * (h + 1), QC * qc + j0:QC * (qc + 1)],
                        start=True, stop=True,
                    )
                pt = ptpool.tile([KB, 2 * QC], bf16, tag="pt")
                if p < 0:
                    nc.scalar.activation(pt[:], ps[:], EXP, scale=1.0 / NORM)
                else:
                    for h in range(2):
                        nc.scalar.activation(
                            pt[:, QC * h + j0:QC * (h + 1)],
                            ps[:, QC * h + j0:QC * (h + 1)],
                            EXP, scale=1.0 / NORM)
                    for h in range(2):
                        # only the leading 128 columns of a diagonal block
                        # straddle the causal boundary; the rest is unmasked
                        nc.vector.tensor_tensor(
                            pt[:, QC * h + j0:QC * h + j0 + KB],
                            pt[:, QC * h + j0:QC * h + j0 + KB],
                            mask_t[0][:, 0:KB],
                            op=AluOpType.mult,
                        )
                for h in range(2):
                    pts[a, qc, h, kb] = (pt, QC * h, j0)

            def emit_pv(a, qc, h, kbs, psy, nkb, state={}):
                for kb in kbs:
                    pt, off, j0 = pts.pop((a, qc, h, kb))
                    i = state.get((a, qc, h), 0)
                    nc.tensor.matmul(
                        psy[0:HD + 1, j0:QC],
                        lhsT=vaug[a % 2][kb][:, (HD + 1) * h:(HD + 1) * (h + 1)],
                        rhs=pt[:, off + j0:off + QC],
                        start=(i == 0), stop=(i == nkb - 1),
                    )
                    state[a, qc, h] = i + 1

            def emit_drain(a, qc, h, psy):
                if (a, qc) not in y_tiles:
                    y_tiles[a, qc] = ypool.tile([FPC, QC], bf16, tag="y", name="y")
                y = y_tiles[a, qc]
                srow = smpool.tile([1, QC], f32, tag=f"srow{h}")
                nc.vector.tensor_copy(srow[:], psy[HD:HD + 1, :])
                yu = smpool.tile([HD, QC], f32, tag=f"yu{h}")
                nc.vector.tensor_copy(yu[:], psy[0:HD, :])
                rec = smpool.tile([1, QC], f32, tag=f"rec{h}")
                nc.vector.reciprocal_approx_fast(rec[:], srow[:])
                bc = bcpool.tile([HD, QC], f32, tag=f"bc{h}")
                nc.gpsimd.partition_broadcast(bc[:], rec[:])
                nc.vector.tensor_tensor(y[HD * h:HD * (h + 1), :], yu[:],
                                        bc[:], op=AluOpType.mult)

            def emit_proj(a, qc, mts):
                y = y_tiles[a, qc]
                for mt in mts:
                    pso = psO.tile([128, QC], f32, tag="pso")
                    nc.tensor.matmul(
                        pso[:],
                        lhsT=wp_t[:, 128 * mt:128 * (mt + 1)],
                        rhs=y[:],
                        start=True, stop=True,
                    )
                    ot = otpool.tile([128, QC], bf16, tag="ot")
                    nc.vector.tensor_copy(ot[:], pso[:])
                    nc.sync.dma_start(
                        out[a, 128 * mt:128 * (mt + 1), QC * qc:QC * (qc + 1)],
                        ot[:],
                    )

            def kb_order(qc):
                nkb = 4 * (qc + 1)
                return ([kb for kb in range(nkb) if kb < 4 * qc] +
                        [kb for kb in range(nkb) if kb >= 4 * qc])

            def attn_units(a):
                # ordered attention stream for batch a; yields callables.
                units = []

                def slot_u(qc, kb):
                    units.append(lambda: emit_slot(a, qc, kb))

                psy_tiles = {}

                def pv_u(qc, h, kbs):
                    def f():
                        if (qc, h) not in psy_tiles:
                            psy_tiles[qc, h] = psY.tile([HD + 1, QC], f32,
                                                        tag="psy", name="psy")
                        emit_pv(a, qc, h, kbs, psy_tiles[qc, h], 4 * (qc + 1))
                    units.append(f)

                def drain_u(qc, h):
                    units.append(lambda: emit_drain(a, qc, h, psy_tiles[qc, h]))

                def proj_u(qc, mts):
                    units.append(lambda: emit_proj(a, qc, list(mts)))

                # scores for chunks 0 and 1 feed ACT early (these land
                # interleaved into QKV(a+1) via merge())
                for kb in range(4):
                    slot_u(0, kb)
                for kb in range(8):
                    slot_u(1, kb)
                ko0, ko1 = kb_order(0), kb_order(1)
                ko2, ko3 = kb_order(2), kb_order(3)
                pv_u(0, 0, ko0[:2]); slot_u(2, 0)
                pv_u(0, 1, ko0[:2]); slot_u(2, 1)
                pv_u(0, 0, ko0[2:]); drain_u(0, 0); slot_u(2, 2)
                pv_u(0, 1, ko0[2:]); drain_u(0, 1); slot_u(2, 3)
                slot_u(2, 4); pv_u(1, 0, ko1[:3])
                slot_u(2, 5); pv_u(1, 1, ko1[:3])
                slot_u(2, 6); pv_u(1, 0, ko1[3:6])
                slot_u(2, 7); pv_u(1, 1, ko1[3:6])
                slot_u(2, 8); pv_u(1, 0, ko1[6:]); drain_u(1, 0)
                slot_u(2, 9); pv_u(1, 1, ko1[6:]); drain_u(1, 1)
                slot_u(2, 10); proj_u(0, [0, 1])
                slot_u(2, 11); proj_u(0, [2, 3])
                slot_u(3, 0); proj_u(0, [4, 5])
                slot_u(3, 1); proj_u(0, [6, 7])
                slot_u(3, 2); pv_u(2, 0, ko2[:3])
                slot_u(3, 3); pv_u(2, 1, ko2[:3])
                slot_u(3, 4); pv_u(2, 0, ko2[3:6])
                slot_u(3, 5); pv_u(2, 1, ko2[3:6])
                slot_u(3, 6); pv_u(2, 0, ko2[6:9])
                slot_u(3, 7); pv_u(2, 1, ko2[6:9])
                slot_u(3, 8); pv_u(2, 0, ko2[9:]); drain_u(2, 0)
                slot_u(3, 9); pv_u(2, 1, ko2[9:]); drain_u(2, 1)
                slot_u(3, 10); proj_u(1, [0, 1])
                slot_u(3, 11); proj_u(1, [2, 3])
                slot_u(3, 12); proj_u(1, [4, 5])
                slot_u(3, 13); proj_u(1, [6, 7])
                slot_u(3, 14); pv_u(3, 0, ko3[:3])
                slot_u(3, 15); pv_u(3, 1, ko3[:3])
                pv_u(3, 0, ko3[3:6]); proj_u(2, [0, 1])
                pv_u(3, 1, ko3[3:6]); proj_u(2, [2, 3])
                pv_u(3, 0, ko3[6:9]); proj_u(2, [4, 5])
                pv_u(3, 1, ko3[6:9]); proj_u(2, [6, 7])
                pv_u(3, 0, ko3[9:12])
                pv_u(3, 1, ko3[9:12])
                pv_u(3, 0, ko3[12:]); drain_u(3, 0)
                pv_u(3, 1, ko3[12:]); drain_u(3, 1)
                # proj(3) deferred to the next section
                return units

            def qkv_units(b):
                units = []
                for c in range(NQC):
                    units.append(lambda c=c: emit_qk_part(b, c, 0))
                    units.append(lambda c=c: emit_v_part(b, c, 0))
                    units.append(lambda c=c: emit_qk_part(b, c, 1))
                    units.append(lambda c=c: emit_v_part(b, c, 1))
                return units

            def merge(fill, attn):
                # interleave: lead with 2 fillers, then spread the rest
                # evenly through the attention stream.
                seq = []
                lead = fill[:2]
                rest = fill[2:]
                seq += lead
                if not attn:
                    return seq + rest
                if not rest:
                    return seq + attn
                stride = max(1, len(attn) // len(rest))
                ai = 0
                for i, f in enumerate(rest):
                    nxt = min(len(attn), (i + 1) * stride)
                    seq += attn[ai:nxt]
                    seq.append(f)
                    ai = nxt
                seq += attn[ai:]
                return seq

            # ---- sections ----
            prefetch_x(0)
            prefetch_x(1)
            for b in range(B + 1):
                if b >= 2 and b < B + 1:
                    pass
                fill = []
                if b < B:
                    fill += qkv_units(b)
                if b >= 2:
                    a2 = b - 2
                    fill.append(lambda a2=a2: emit_proj(a2, 3, [0, 1, 2, 3]))
                    fill.append(lambda a2=a2: emit_proj(a2, 3, [4, 5, 6, 7]))
                attn = attn_units(b - 1) if b >= 1 else []
                for u in merge(fill, attn):
                    u()
                if b + 2 <= B - 1:
                    prefetch_x(b + 2)
            # tail: proj(B-1, 3)
            emit_proj(B - 1, 3, list(range(8)))

    nc.compile()
    return nc


def _get_nc():
    if "nc" not in _cache:
        _cache["nc"] = _build()
    return _cache["nc"]


def _make_masks():
    i = np.arange(KB)[:, None]
    j = np.arange(QC)[None, :]
    m = np.zeros((4, KB, QC), dtype=np.float32)
    for p in range(4):
        m[p] = (j >= (KB * p + i)).astype(np.float32)
    return m.astype(_BF16)


def shard_inputs(x, w_qkv, w_proj):
    xt = np.ascontiguousarray(np.asarray(x, dtype=np.float32).transpose(0, 2, 1))
    xt = xt.astype(_BF16)
    w_qkv = np.asarray(w_qkv, dtype=np.float32)
    w_proj = np.asarray(w_proj, dtype=np.float32)
    masks = _make_masks()
    in_maps = []
    for c in range(N_CORES):
        qcols = slice(FPC * c, FPC * (c + 1))
        kcols = slice(D + FPC * c, D + FPC * (c + 1))
        vcols = slice(2 * D + FPC * c, 2 * D + FPC * (c + 1))
        w3_c = np.concatenate(
            [w_qkv[:, qcols], w_qkv[:, kcols], w_qkv[:, vcols]], axis=1)
        in_maps.append({
            "xt": xt,
            "w3": np.ascontiguousarray(w3_c).astype(_BF16),
            "wp": np.ascontiguousarray(w_proj[FPC * c:FPC * (c + 1), :]).astype(_BF16),
            "masks": masks,
        })
    return in_maps


def unshard(results):
    total = results[0]["out"].astype(np.float32)
    for r in results[1:]:
        total += r["out"].astype(np.float32)
    return np.ascontiguousarray(total.transpose(0, 2, 1))


def run(inputs, trace=False, **kw):
    from concourse.bass_utils import run_bass_kernel_spmd

    nc = _get_nc()
    in_maps = shard_inputs(inputs["x"], inputs["w_qkv"], inputs["w_proj"])
    res = run_bass_kernel_spmd(nc, in_maps, core_ids=list(range(N_CORES)),
                               trace=trace, **kw)
    return unshard(res.results), res


def kernel(**inputs):
    out, _ = run(inputs, trace=False)
    return out


# revision 20
# speedup vs baseline: 1.8270x; 1.0234x over previous
"""Multi-head causal attention (B=4, T=2048, D=1024, H=16) on 8 TRN2 cores.

Tensor-parallel over heads: core c computes heads {2c, 2c+1}. Redesign vs
the previous version:
  - Score matmuls for h0 (PE rows 0-63) and h1 (rows 64-127) are emitted
    back-to-back into separate PSUM banks so the row-tiled pairs execute
    concurrently (~2x score throughput).
  - V is computed directly token-major (lhsT = x^T token slice), so there
    are no PE transposes and no transpose->copy->memset chain.
  - vaug ones-columns are persistent tiles memset once at startup (the old
    per-batch gpsimd memsets stalled PV ~30us/batch behind the norm chain).
  - Norm chain: DVE copy of the denominator row, reciprocal_approx_fast,
    gpsimd partition_broadcast, DVE multiply into y (bf16).
  - Proj drains on DVE as bf16; output DMA'd bf16 and summed on host.
  - Emission is software-pipelined: section b emits QKV(b) interleaved with
    attention of batch b-1 (scores qc0/1 during QKV, PV/proj later), and
    proj(b-1, qc3) is deferred into section b+1.
"""

import sys

for _p in ("/opt/trn_rl_repo",):
    if _p not in sys.path:
        sys.path.append(_p)

import numpy as np
import ml_dtypes

B, T, D = 4, 2048, 1024
H = 16
HD = D // H
NORM = float(np.sqrt(D))
N_CORES = 8
HEADS_PER_CORE = H // N_CORES          # 2
FPC = HEADS_PER_CORE * HD              # 128 features per core
QC = 512                               # query chunk
NQC = T // QC                          # 4
KB = 128                               # key block
DKC = D // 128                         # 8 contraction chunks over D
NTB = T // 128                         # 16 token blocks

_BF16 = ml_dtypes.bfloat16

_cache = {}

N_WARM = 180


def _build():
    import concourse.bacc as bacc
    import concourse.mybir as mybir
    from concourse.tile import TileContext
    from concourse.alu_op_type import AluOpType

    f32 = mybir.dt.float32
    bf16 = mybir.dt.bfloat16
    EXP = mybir.ActivationFunctionType.Exp

    nc = bacc.Bacc("TRN2", target_bir_lowering=False, debug=False,
                   num_devices=N_CORES)

    xt = nc.dram_tensor("xt", [B, D, T], bf16, kind="ExternalInput").ap()
    w3 = nc.dram_tensor("w3", [D, 3 * FPC], bf16, kind="ExternalInput").ap()
    wp = nc.dram_tensor("wp", [FPC, D], bf16, kind="ExternalInput").ap()
    masks = nc.dram_tensor("masks", [4, KB, QC], bf16, kind="ExternalInput").ap()
    out = nc.dram_tensor("out", [B, D, T], bf16, kind="ExternalOutput").ap()

    with TileContext(nc) as tc:
        with (
            tc.tile_pool(name="const", bufs=1) as cpool,
            tc.tile_pool(name="xp", bufs=12) as xpool,
            tc.tile_pool(name="qk", bufs=2) as qkpool,
            tc.tile_pool(name="pt", bufs=30) as ptpool,
            tc.tile_pool(name="y", bufs=6) as ypool,
            tc.tile_pool(name="sm", bufs=3) as smpool,
            tc.tile_pool(name="bcp", bufs=2) as bcpool,
            tc.tile_pool(name="ot", bufs=4) as otpool,
            tc.tile_pool(name="psA", bufs=2, space="PSUM") as psA,
            tc.tile_pool(name="psY", bufs=2, space="PSUM") as psY,
            tc.tile_pool(name="psO", bufs=2, space="PSUM") as psO,
        ):
            # ---- constants ----
            w3_t = []
            for kc in range(DKC):
                t = cpool.tile([128, 3 * FPC], bf16, tag=f"w3{kc}")
                nc.sync.dma_start(t[:], w3[kc * 128:(kc + 1) * 128, :])
                w3_t.append(t)
            wp_t = cpool.tile([FPC, D], bf16, tag="wp")
            nc.sync.dma_start(wp_t[:], wp[:])
            mask_t = []
            for p in range(4):
                t = cpool.tile([KB, QC], bf16, tag=f"mask{p}")
                nc.sync.dma_start(t[:], masks[p])
                mask_t.append(t)

            # persistent vaug tiles: [v_h0 64 | ones | v_h1 64 | ones],
            # double-buffered across batches; ones written once here.
            vaug = [[], []]
            for g in range(2):
                for tb in range(NTB):
                    va = cpool.tile([128, 2 * HD + 2], bf16, tag=f"va{g}_{tb}")
                    nc.gpsimd.memset(va[:, HD:HD + 1], 1.0)
                    nc.gpsimd.memset(va[:, 2 * HD + 1:2 * HD + 2], 1.0)
                    vaug[g].append(va)

            # PE warmup on a memset tile (no DMA dependency): keeps the HAM
            # clock-gate busy during the initial x DMA.
            wt = cpool.tile([128, 128], bf16, tag="warm")
            nc.vector.memset(wt[:], 0.25)
            psw = psO.tile([128, QC], f32, tag="pso")
            for _ in range(N_WARM):
                nc.tensor.matmul(psw[:, 0:128], lhsT=wt[:], rhs=wt[:],
                                 start=True, stop=True)

            # ---- mutable cross-section state ----
            xp_t = {}      # b -> [8 tiles]
            qkp = {}       # b -> (qp, kp)
            pts = {}       # (a, qc, h, kb) -> (pt tile, j0)
            y_tiles = {}   # (a, qc) -> y tile

            def prefetch_x(b):
                ts = []
                for kc in range(DKC):
                    t = xpool.tile([128, T], bf16, tag="xp")
                    nc.sync.dma_start(t[:], xt[b, kc * 128:(kc + 1) * 128, :])
                    ts.append(t)
                xp_t[b] = ts

            # ---- QKV units ----
            # Each chunk c is emitted as two interleaved parts so the short
            # N=128 token-major V matmuls hide their weight loads under the
            # long N=512 Q/K streams:
            #   part 0: Q[kc] + V(tb 4c+0)[kc] + V(tb 4c+1)[kc]  for kc=0..7
            #   part 1: K[kc] + V(tb 4c+2)[kc] + V(tb 4c+3)[kc]
            def emit_qk_part(b, c, part):
                if b not in qkp:
                    qp = qkpool.tile([128, T], bf16, tag="qp")
                    kp = qkpool.tile([128, T], bf16, tag="kp")
                    qkp[b] = (qp, kp)
                dst = qkp[b][part]
                ps = psO.tile([128, QC], f32, tag="pso", name="psqk")
                for kc in range(DKC):
                    nc.tensor.matmul(
                        ps[:],
                        lhsT=w3_t[kc][:, 128 * part:128 * (part + 1)],
                        rhs=xp_t[b][kc][:, QC * c:QC * (c + 1)],
                        start=(kc == 0), stop=(kc == DKC - 1),
                    )
                nc.vector.tensor_copy(dst[:, QC * c:QC * (c + 1)], ps[:])

            def emit_v_part(b, c, part):
                psv = psO.tile([128, QC], f32, tag="pso", name="psv")
                tbs = (4 * c + 2 * part, 4 * c + 2 * part + 1)
                for j, tb in enumerate(tbs):
                    for kc in range(DKC):
                        nc.tensor.matmul(
                            psv[:, 256 * j:256 * j + 128],
                            lhsT=xp_t[b][kc][:, 128 * tb:128 * (tb + 1)],
                            rhs=w3_t[kc][:, 256:384],
                            start=(kc == 0), stop=(kc == DKC - 1),
                        )
                for j, tb in enumerate(tbs):
                    va = vaug[b % 2][tb]
                    nc.vector.tensor_copy(va[:, 0:HD],
                                          psv[:, 256 * j:256 * j + HD])
                    nc.vector.tensor_copy(va[:, HD + 1:2 * HD + 1],
                                          psv[:, 256 * j + HD:256 * j + 128])

            # ---- attention units ----
            def emit_slot(a, qc, kb):
                # one key block, both heads: two concurrent row-tiled MMs
                # into the two banks of one PSUM pair tile, exp on ACT,
                # triangular-boundary mask on DVE.
                j0 = max(0, KB * (kb - 4 * qc))
                p = kb - 4 * qc
                qp, kp = qkp[a]
                ps = psA.tile([128, 2 * QC], f32, tag="ps")
                for h in range(2):
                    nc.tensor.matmul(
                        ps[:, QC * h + j0:QC * (h + 1)],
                        lhsT=kp[HD * h:HD * (h + 1), KB * kb:KB * (kb + 1)],
                        rhs=qp[HD * h:HD * (h + 1), QC * qc + j0:QC * (qc + 1)],
                        start=True, stop=True,
                    )
                pt = ptpool.tile([KB, 2 * QC], bf16, tag="pt")
                if p < 0:
                    nc.scalar.activation(pt[:], ps[:], EXP, scale=1.0 / NORM)
                else:
                    for h in range(2):
                        nc.scalar.activation(
                            pt[:, QC * h + j0:QC * (h + 1)],
                            ps[:, QC * h + j0:QC * (h + 1)],
                            EXP, scale=1.0 / NORM)
                    for h in range(2):
                        # only the leading 128 columns of a diagonal block
                        # straddle the causal boundary; the rest is unmasked
                        nc.vector.tensor_tensor(
                            pt[:, QC * h + j0:QC * h + j0 + KB],
                            pt[:, QC * h + j0:QC * h + j0 + KB],
                            mask_t[0][:, 0:KB],
                            op=AluOpType.mult,
                        )
                for h in range(2):
                    pts[a, qc, h, kb] = (pt, QC * h, j0)

            def emit_pv(a, qc, h, kbs, psy, nkb, state={}):
                for kb in kbs:
                    pt, off, j0 = pts.pop((a, qc, h, kb))
                    i = state.get((a, qc, h), 0)
                    nc.tensor.matmul(
                        psy[0:HD + 1, j0:QC],
                        lhsT=vaug[a % 2][kb][:, (HD + 1) * h:(HD + 1) * (h + 1)],
                        rhs=pt[:, off + j0:off + QC],
                        start=(i == 0), stop=(i == nkb - 1),
                    )
                    state[a, qc, h] = i + 1

            def emit_drain(a, qc, h, psy):
                if (a, qc) not in y_tiles:
                    y_tiles[a, qc] = ypool.tile([FPC, QC], bf16, tag="y", name="y")
                y = y_tiles[a, qc]
                srow = smpool.tile([1, QC], f32, tag=f"srow{h}")
                yu = smpool.tile([HD, QC], f32, tag=f"yu{h}")
                if qc == 3:
                    # chunk-3 drains run when ACT has no exp work left
                    nc.scalar.copy(srow[:], psy[HD:HD + 1, :])
                    nc.scalar.copy(yu[:], psy[0:HD, :])
                else:
                    nc.vector.tensor_copy(srow[:], psy[HD:HD + 1, :])
                    nc.vector.tensor_copy(yu[:], psy[0:HD, :])
                rec = smpool.tile([1, QC], f32, tag=f"rec{h}")
                nc.vector.reciprocal_approx_fast(rec[:], srow[:])
                bc = bcpool.tile([HD, QC], f32, tag=f"bc{h}")
                nc.gpsimd.partition_broadcast(bc[:], rec[:])
                nc.vector.tensor_tensor(y[HD * h:HD * (h + 1), :], yu[:],
                                        bc[:], op=AluOpType.mult)

            def emit_proj(a, qc, mts):
                y = y_tiles[a, qc]
                for mt in mts:
                    pso = psO.tile([128, QC], f32, tag="pso")
                    nc.tensor.matmul(
                        pso[:],
                        lhsT=wp_t[:, 128 * mt:128 * (mt + 1)],
                        rhs=y[:],
                        start=True, stop=True,
                    )
                    ot = otpool.tile([128, QC], bf16, tag="ot")
                    if qc == 3:
                        # chunk-3 proj drains overlap the next section's
                        # QKV phase, where ACT is otherwise idle
                        nc.scalar.copy(ot[:], pso[:])
                    else:
                        nc.vector.tensor_copy(ot[:], pso[:])
                    nc.sync.dma_start(
                        out[a, 128 * mt:128 * (mt + 1), QC * qc:QC * (qc + 1)],
                        ot[:],
                    )

            def kb_order(qc):
                nkb = 4 * (qc + 1)
                return ([kb for kb in range(nkb) if kb < 4 * qc] +
                        [kb for kb in range(nkb) if kb >= 4 * qc])

            def attn_units(a):
                # ordered attention stream for batch a; yields callables.
                units = []

                def slot_u(qc, kb):
                    units.append(lambda: emit_slot(a, qc, kb))

                psy_tiles = {}

                def pv_u(qc, h, kbs):
                    def f():
                        if (qc, h) not in psy_tiles:
                            psy_tiles[qc, h] = psY.tile([HD + 1, QC], f32,
                                                        tag="psy", name="psy")
                        emit_pv(a, qc, h, kbs, psy_tiles[qc, h], 4 * (qc + 1))
                    units.append(f)

                def drain_u(qc, h):
                    units.append(lambda: emit_drain(a, qc, h, psy_tiles[qc, h]))

                def proj_u(qc, mts):
                    units.append(lambda: emit_proj(a, qc, list(mts)))

                # scores for chunks 0 and 1 feed ACT early (these land
                # interleaved into QKV(a+1) via merge())
                for kb in range(4):
                    slot_u(0, kb)
                for kb in range(8):
                    slot_u(1, kb)
                ko0, ko1 = kb_order(0), kb_order(1)
                ko2, ko3 = kb_order(2), kb_order(3)
                pv_u(0, 0, ko0[:2]); slot_u(2, 0)
                pv_u(0, 1, ko0[:2]); slot_u(2, 1)
                pv_u(0, 0, ko0[2:]); drain_u(0, 0); slot_u(2, 2)
                pv_u(0, 1, ko0[2:]); drain_u(0, 1); slot_u(2, 3)
                slot_u(2, 4); pv_u(1, 0, ko1[:3])
                slot_u(2, 5); pv_u(1, 1, ko1[:3])
                slot_u(2, 6); pv_u(1, 0, ko1[3:6])
                slot_u(2, 7); pv_u(1, 1, ko1[3:6])
                slot_u(2, 8); pv_u(1, 0, ko1[6:]); drain_u(1, 0)
                slot_u(2, 9); pv_u(1, 1, ko1[6:]); drain_u(1, 1)
                slot_u(2, 10); proj_u(0, [0, 1])
                slot_u(2, 11); proj_u(0, [2, 3])
                slot_u(3, 0); proj_u(0, [4, 5])
                slot_u(3, 1); proj_u(0, [6, 7])
                slot_u(3, 2); pv_u(2, 0, ko2[:3])
                slot_u(3, 3); pv_u(2, 1, ko2[:3])
                slot_u(3, 4); pv_u(2, 0, ko2[3:6])
                slot_u(3, 5); pv_u(2, 1, ko2[3:6])
                slot_u(3, 6); pv_u(2, 0, ko2[6:9])
                slot_u(3, 7); pv_u(2, 1, ko2[6:9])
                slot_u(3, 8); pv_u(2, 0, ko2[9:]); drain_u(2, 0)
                slot_u(3, 9); pv_u(2, 1, ko2[9:]); drain_u(2, 1)
                slot_u(3, 10); proj_u(1, [0, 1])
                slot_u(3, 11); proj_u(1, [2, 3])
                slot_u(3, 12); proj_u(1, [4, 5])
                slot_u(3, 13); proj_u(1, [6, 7])
                slot_u(3, 14); pv_u(3, 0, ko3[:3])
                slot_u(3, 15); pv_u(3, 1, ko3[:3])
                pv_u(3, 0, ko3[3:6]); proj_u(2, [0, 1])
                pv_u(3, 1, ko3[3:6]); proj_u(2, [2, 3])
                pv_u(3, 0, ko3[6:9]); proj_u(2, [4, 5])
                pv_u(3, 1, ko3[6:9]); proj_u(2, [6, 7])
                pv_u(3, 0, ko3[9:12])
                pv_u(3, 1, ko3[9:12])
                pv_u(3, 0, ko3[12:]); drain_u(3, 0)
                pv_u(3, 1, ko3[12:]); drain_u(3, 1)
                # proj(3) deferred to the next section
                return units

            def qkv_units(b):
                units = []
                for c in range(NQC):
                    units.append(lambda c=c: emit_qk_part(b, c, 0))
                    units.append(lambda c=c: emit_v_part(b, c, 0))
                    units.append(lambda c=c: emit_qk_part(b, c, 1))
                    units.append(lambda c=c: emit_v_part(b, c, 1))
                return units

            def merge(fill, attn):
                # interleave: lead with 2 fillers, then spread the rest
                # evenly through the attention stream.
                seq = []
                lead = fill[:2]
                rest = fill[2:]
                seq += lead
                if not attn:
                    return seq + rest
                if not rest:
                    return seq + attn
                stride = max(1, len(attn) // len(rest))
                ai = 0
                for i, f in enumerate(rest):
                    nxt = min(len(attn), (i + 1) * stride)
                    seq += attn[ai:nxt]
                    seq.append(f)
                    ai = nxt
                seq += attn[ai:]
                return seq

            # ---- sections ----
            prefetch_x(0)
            prefetch_x(1)
            for b in range(B + 1):
                if b >= 2 and b < B + 1:
                    pass
                fill = []
                if b < B:
                    fill += qkv_units(b)
                if b >= 2:
                    a2 = b - 2
                    fill.append(lambda a2=a2: emit_proj(a2, 3, [0, 1, 2, 3]))
                    fill.append(lambda a2=a2: emit_proj(a2, 3, [4, 5, 6, 7]))
                attn = attn_units(b - 1) if b >= 1 else []
                for u in merge(fill, attn):
                    u()
                if b + 2 <= B - 1:
                    prefetch_x(b + 2)
            # tail: proj(B-1, 3)
            emit_proj(B - 1, 3, list(range(8)))

    nc.compile()
    return nc


def _get_nc():
    if "nc" not in _cache:
        _cache["nc"] = _build()
    return _cache["nc"]


def _make_masks():
    i = np.arange(KB)[:, None]
    j = np.arange(QC)[None, :]
    m = np.zeros((4, KB, QC), dtype=np.float32)
    for p in range(4):
        m[p] = (j >= (KB * p + i)).astype(np.float32)
    return m.astype(_BF16)


def shard_inputs(x, w_qkv, w_proj):
    xt = np.ascontiguousarray(np.asarray(x, dtype=np.float32).transpose(0, 2, 1))
    xt = xt.astype(_BF16)
    w_qkv = np.asarray(w_qkv, dtype=np.float32)
    w_proj = np.asarray(w_proj, dtype=np.float32)
    masks = _make_masks()
    in_maps = []
    for c in range(N_CORES):
        qcols = slice(FPC * c, FPC * (c + 1))
        kcols = slice(D + FPC * c, D + FPC * (c + 1))
        vcols = slice(2 * D + FPC * c, 2 * D + FPC * (c + 1))
        w3_c = np.concatenate(
            [w_qkv[:, qcols], w_qkv[:, kcols], w_qkv[:, vcols]], axis=1)
        in_maps.append({
            "xt": xt,
            "w3": np.ascontiguousarray(w3_c).astype(_BF16),
            "wp": np.ascontiguousarray(w_proj[FPC * c:FPC * (c + 1), :]).astype(_BF16),
            "masks": masks,
        })
    return in_maps


def unshard(results):
    total = results[0]["out"].astype(np.float32)
    for r in results[1:]:
        total += r["out"].astype(np.float32)
    return np.ascontiguousarray(total.transpose(0, 2, 1))


def run(inputs, trace=False, **kw):
    from concourse.bass_utils import run_bass_kernel_spmd

    nc = _get_nc()
    in_maps = shard_inputs(inputs["x"], inputs["w_qkv"], inputs["w_proj"])
    res = run_bass_kernel_spmd(nc, in_maps, core_ids=list(range(N_CORES)),
                               trace=trace, **kw)
    return unshard(res.results), res


def kernel(**inputs):
    out, _ = run(inputs, trace=False)
    return out
